# revision 6
# baseline (speedup 1.0000x reference)
"""Trainium2 Bass kernel for nn_CustomGPM (multi-scale temporal CNN + RGCN + actor head).

v3: bf16 datapath, DMA spread over all 5 engine queues with per-relation
adjacency chunks, DMA-independent PE warmup + scalar table priming,
pair-packed conv psums/activations, col-tiled RGCN aggregation, and a
DMA-free z-transpose tail.

Layout per core (BL=8 batch elems, 4 pairs):
  xsml[j] [67, 1024] bf16: rows 0:20 s-conv2, 20:40 m-conv2, 40:64 zero,
    64:67 l(max_t); cols b0 0:500, b1 512:1012 (bank-aligned regions)
  hsb[b][i] [125, 344] bf16: node-on-partition H = x^T W_rel, chunks 2i,2i+1
  agg psum [107, 500]: rows 0:43 b0 feats, 64:107 b1 (col-tiled matmuls)
"""

import numpy as np
import ml_dtypes

BF = ml_dtypes.bfloat16

# ---------------- problem constants (hardcoded per spec) ----------------
B = 64
NCORES = 8
BL = B // NCORES          # 8 per core, 4 pairs
C0, N, T, R, P, H = 3, 500, 50, 4, 500, 128
CF = 20
F = 2 * CF + C0           # 43
NCH = 125
TS1, TM1 = 48, 30
SLOPE = 0.01
EPS = 1e-5

# packA (bf16, 128 partitions) column offsets
OA_W1 = 0                 # [150 -> 128+22 split, 234] merged conv1 band
OA_C2K1 = 234             # [128, 40]
OA_AW2 = OA_C2K1 + 40     # [128, 128]
OA_WALL = OA_AW2 + 128    # [67, 172]
OA_WROOT = OA_WALL + 172  # [67, 43]
OA_WZPT = OA_WROOT + 43   # [67, 1]
OA_WZPT2 = OA_WZPT + 1    # [67, 2]
OA_WZG2 = OA_WZPT2 + 2    # [107, 2]
OA_B3R = OA_WZG2 + 2      # [1, 501]
OA_ONES = OA_B3R + 501    # [1, 8]
OA_RH = OA_ONES + 8       # [128, 4000] obs rows 0:128, cols b*500+n
CA = OA_RH + 4000

# pack22 (bf16, 22 partitions)
O22_W1 = 0                # [22, 234] conv1 band rows 128:150
O22_RL = 234              # [22, 4000] obs rows 128:150
C22 = O22_RL + 4000

# pdsmall (bf16, 106 partitions): conv2 K-tile2 + identity
OPS_C2K2 = 0              # [106, 40]
OPS_ID8 = 40              # [8, 8] identity
CPS = 48

# wtail (bf16, 128 partitions): tail-phase weights
OT_AW3 = 0                # [128, 501]
OT_W1C = 501              # [125, 1024] fc1 chunks
OT_ATS = OT_W1C + 1024    # [125, 32] action^T (c, b)
CT = OT_ATS + 32

_CACHE = {}


# ======================= host-side parameter folding =======================

def _bn_fold(p):
    g, b, m, v = np.asarray(p, np.float64)
    s = g / np.sqrt(v + EPS)
    return s, b - m * s


def _conv_band_lhsT(w, bias, bn, t_out):
    w = np.asarray(w, np.float64)[:, :, 0, :]
    co, ci, k = w.shape
    s, t_ = _bn_fold(bn)
    w_eff = w * s[:, None, None]
    b_eff = s * np.asarray(bias, np.float64) + t_
    band = np.zeros((co, t_out, ci, T), np.float64)
    for t in range(t_out):
        band[:, t, :, t:t + k] = w_eff
    lhsT = band.reshape(co * t_out, ci * T).T.copy()
    return lhsT, np.repeat(b_eff, t_out)


def _conv2_fold(w, b, bn):
    w = np.asarray(w, np.float64)[:, :, 0, :]
    s, t_ = _bn_fold(bn)
    w_eff = (w * s[:, None, None]).reshape(CF, -1)
    b_eff = s * np.asarray(b, np.float64) + t_
    return w_eff.T.copy(), b_eff


def _pad67(a):
    """[43, X] -> [67, X]: rows 0:40 = a[0:40], 64:67 = a[40:43]."""
    out = np.zeros((67,) + a.shape[1:], np.float64)
    out[0:40] = a[0:40]
    out[64:67] = a[40:43]
    return out


def _host_fold(inp):
    ws1, bs1 = _conv_band_lhsT(inp['sc1_w'], inp['sc1_b'], inp['sbn1'], TS1)
    wm1, bm1 = _conv_band_lhsT(inp['mc1_w'], inp['mc1_b'], inp['mbn1'], TM1)
    w1all = np.concatenate([ws1, wm1], axis=1)              # [150, 234]
    bias_a = bs1[0:128]
    bias_b = np.concatenate([bs1[128:144], bm1])            # [106]

    ws2, bs2 = _conv2_fold(inp['sc2_w'], inp['sc2_b'], inp['sbn2'])  # [144,20]
    wm2, bm2 = _conv2_fold(inp['mc2_w'], inp['mc2_b'], inp['mbn2'])  # [90,20]
    c2k1 = np.zeros((128, 40), np.float64)
    c2k1[:, 0:20] = ws2[0:128]
    c2k2 = np.zeros((106, 40), np.float64)
    c2k2[0:16, 0:20] = ws2[128:144]
    c2k2[16:106, 20:40] = wm2
    bias_c = np.concatenate([bs2, bm2])                     # [40]

    sg, tg = _bn_fold(inp['gbn'])
    w_all = np.concatenate(
        [np.asarray(inp['gw_rel'], np.float64)[r] * sg[None, :]
         for r in range(R)], axis=1)                        # [43, 172]
    w_root = np.asarray(inp['gw_root'], np.float64) * sg[None, :]
    gb_eff = np.asarray(inp['g_b'], np.float64) * sg + tg
    wallt = _pad67(w_all)
    wroott = _pad67(w_root)

    src = np.asarray(inp['edge_index'][0]).astype(np.int64)
    dst = np.asarray(inp['edge_index'][1]).astype(np.int64)
    etype = np.asarray(inp['edge_type']).astype(np.int64)
    a_t = np.zeros((R, N, N), np.float64)                   # [r, src, dst]
    for r in range(R):
        sel = etype == r
        cnt = np.zeros((N, N), np.float64)
        np.add.at(cnt, (dst[sel], src[sel]), 1.0)
        deg = cnt.sum(axis=1)
        a_t[r] = (cnt / np.maximum(deg, 1.0)[:, None]).T
    # per relation: [125, (c, n)] with src chunked on partitions
    attr = [np.ascontiguousarray(
        a_t[r].reshape(4, NCH, N).transpose(1, 0, 2).reshape(NCH, 4 * N)
    ).astype(BF) for r in range(R)]

    a_cw = np.asarray(inp['a_cw'], np.float64)
    a_cb = float(np.asarray(inp['a_cb'], np.float64)[0])
    a_w1 = np.asarray(inp['a_w1'], np.float64)
    sel_nodes = np.asarray(inp['nodes_to_select']).astype(np.int64)
    w_z = a_cw[1:1 + 2 * F]
    wzpt = _pad67(w_z[0:F].reshape(F, 1))
    wzpt2 = np.zeros((67, 2), np.float64)
    wzpt2[:, 1:2] = wzpt
    wzg2 = np.zeros((107, 2), np.float64)
    wzg2[0:43, 0] = w_z[F:]
    wzg2[64:107, 1] = w_z[F:]
    col_g = np.zeros(107, np.float64)
    col_g[0:43] = gb_eff
    col_g[64:107] = gb_eff

    w1z = np.zeros((N, H), np.float64)
    np.add.at(w1z, sel_nodes, a_w1[1:])
    w1a = a_cw[0] * a_w1[1:]
    b1_eff = np.asarray(inp['a_b1'], np.float64) + a_cb * a_w1[1:].sum(axis=0)
    w1cat = np.concatenate([w1z, w1a], axis=0)              # [1000, 128]
    w1c = w1cat.reshape(8, NCH, H).transpose(1, 0, 2).reshape(NCH, 8 * H)

    biasf = np.zeros((128, 6 + NCH), np.float32)
    biasf[0:128, 0] = bias_a
    biasf[0:106, 1] = bias_b
    biasf[0:40, 2] = bias_c
    biasf[0:107, 3] = col_g
    biasf[0:128, 4] = b1_eff
    biasf[0:128, 5] = np.asarray(inp['a_b2'], np.float64)
    biasf[0:NCH, 6:6 + NCH] = np.eye(NCH)                   # f32 transpose id

    pa = np.zeros((128, OA_RH), np.float64)
    pa[:, OA_W1:OA_W1 + 234] = w1all[0:128]
    pa[:, OA_C2K1:OA_C2K1 + 40] = c2k1
    pa[:, OA_AW2:OA_AW2 + 128] = np.asarray(inp['a_w2'], np.float64)
    pa[0:67, OA_WALL:OA_WALL + 172] = wallt
    pa[0:67, OA_WROOT:OA_WROOT + 43] = wroott
    pa[0:67, OA_WZPT:OA_WZPT + 1] = wzpt
    pa[0:67, OA_WZPT2:OA_WZPT2 + 2] = wzpt2
    pa[0:107, OA_WZG2:OA_WZG2 + 2] = wzg2
    pa[0:1, OA_B3R:OA_B3R + 501] = np.asarray(inp['a_b3'], np.float64)
    pa[0:1, OA_ONES:OA_ONES + 8] = 1.0

    ps = np.zeros((106, CPS), np.float64)
    ps[0:106, OPS_C2K2:OPS_C2K2 + 40] = c2k2
    ps[0:8, OPS_ID8:OPS_ID8 + 8] = np.eye(8)

    wt = np.zeros((128, CT), np.float64)
    wt[:, OT_AW3:OT_AW3 + 501] = np.asarray(inp['a_w3'], np.float64)
    wt[0:NCH, OT_W1C:OT_W1C + 1024] = w1c

    return {
        'pa_const': pa.astype(BF), 'p22_const': w1all[128:150].astype(BF),
        'ps_const': ps.astype(BF), 'wt_const': wt.astype(BF),
        'attr': attr, 'biasf': biasf,
    }


# ============================ device kernel ============================

def _build_nc():
    import concourse.bacc as bacc
    import concourse.tile as tile
    import concourse.mybir as mybir
    from contextlib import ExitStack

    F32 = mybir.dt.float32
    BF16 = mybir.dt.bfloat16
    AF = mybir.ActivationFunctionType
    ALU = mybir.AluOpType
    AX = mybir.AxisListType

    nc = bacc.Bacc("TRN2", target_bir_lowering=False, debug=False)

    packA_d = nc.dram_tensor('packA', [128, CA], BF16, kind="ExternalInput").ap()
    pack22_d = nc.dram_tensor('pack22', [22, C22], BF16, kind="ExternalInput").ap()
    pds_d = nc.dram_tensor('pdsmall', [106, CPS], BF16, kind="ExternalInput").ap()
    onat0_d = nc.dram_tensor('onat0', [NCH, 2400], BF16, kind="ExternalInput").ap()
    onat1_d = nc.dram_tensor('onat1', [NCH, 2400], BF16, kind="ExternalInput").ap()
    attr_d = [nc.dram_tensor(f'attr{r}', [NCH, 4 * N], BF16,
                             kind="ExternalInput").ap() for r in range(R)]
    wtail_d = nc.dram_tensor('wtail', [128, CT], BF16, kind="ExternalInput").ap()
    biasF_d = nc.dram_tensor('biasF', [128, 6 + NCH], F32,
                             kind="ExternalInput").ap()
    out_d = nc.dram_tensor('out', [BL, P + 1], F32, kind="ExternalOutput").ap()

    mm = nc.tensor.matmul

    with tile.TileContext(nc) as tc, ExitStack() as ctx:
        cp = ctx.enter_context(tc.tile_pool(name="const", bufs=1))
        pw = ctx.enter_context(tc.tile_pool(name="work", bufs=2))
        pp = ctx.enter_context(tc.tile_pool(name="pp", bufs=2, space="PSUM"))

        zw = cp.tile([128, 512], BF16, name='zw', tag='zw')
        nc.gpsimd.memset(zw[:], 0)

        pA = cp.tile([128, CA], BF16, name='pA', tag='pA')
        p22 = cp.tile([22, C22], BF16, name='p22', tag='p22')
        pS = cp.tile([106, CPS], BF16, name='pS', tag='pS')
        on0 = cp.tile([NCH, 2400], BF16, name='on0', tag='on0')
        on1 = cp.tile([NCH, 2400], BF16, name='on1', tag='on1')
        attr = [cp.tile([NCH, 4 * N], BF16, name=f'attr{r}', tag=f'attr{r}')
                for r in range(R)]
        wT = cp.tile([128, CT], BF16, name='wT', tag='wT')
        bF = cp.tile([128, 6 + NCH], F32, name='bF', tag='bF')

        # DMA plan: spread across all five engine queues
        nc.sync.dma_start(out=pA[:], in_=packA_d[:])
        nc.sync.dma_start(out=attr[0][:], in_=attr_d[0][:])
        nc.sync.dma_start(out=attr[1][:], in_=attr_d[1][:])
        nc.scalar.dma_start(out=p22[:], in_=pack22_d[:])
        nc.scalar.dma_start(out=pS[:], in_=pds_d[:])
        nc.scalar.dma_start(out=bF[:], in_=biasF_d[:])
        nc.scalar.dma_start(out=attr[2][:], in_=attr_d[2][:])
        nc.scalar.dma_start(out=attr[3][:], in_=attr_d[3][:])
        nc.gpsimd.dma_start(out=on0[:], in_=onat0_d[:])
        nc.gpsimd.dma_start(out=on1[:], in_=onat1_d[:])
        nc.gpsimd.dma_start(out=wT[:], in_=wtail_d[:])

        xsml = [cp.tile([67, 1024], BF16, name=f'xsml{j}', tag=f'xsml{j}')
                for j in range(4)]
        for j in range(4):
            nc.gpsimd.memset(xsml[j][32:64, :], 0)

        # scalar priming: pull the LEAKY_RELU table load to kernel start
        prim = cp.tile([1, 8], BF16, name='prim', tag='prim')
        nc.scalar.activation(prim[:], zw[0:1, 0:8], AF.Lrelu, alpha=SLOPE)

        # ---- PE warmup on zeros (HAM to K=8/8 while DMAs land) ----
        for w in range(18):
            pwm = pp.tile([128, 512], F32, name=f'pwm{w}', tag='conv')
            mm(pwm[:], zw[:, 0:128], zw[:], start=True, stop=True)

        # ---- l-branch max over t (DVE, early) ----
        lm = []
        for b in range(BL):
            t = cp.tile([NCH, 12], F32, name=f'lm{b}', tag=f'lm{b}')
            src = on0 if b < 4 else on1
            nc.vector.tensor_reduce(
                t[:],
                src[:, (b % 4) * 600:(b % 4 + 1) * 600].rearrange(
                    "p (c k t) -> p c k t", c=4, k=C0),
                axis=AX.X, op=ALU.max)
            lm.append(t)

        # ---- conv pairs (PE + ACT; no l-branch deps here) ----
        for j in range(4):
            psA = pp.tile([128, 1024], F32, name=f'psA{j}', tag='conv')
            psB = pp.tile([106, 1024], F32, name=f'psB{j}', tag='conv')
            for i in range(2):
                b = 2 * j + i
                rh = pA[:, OA_RH + b * N: OA_RH + (b + 1) * N]
                rl = p22[:, O22_RL + b * N: O22_RL + (b + 1) * N]
                co = i * 512
                mm(psA[:, co:co + N], pA[:, OA_W1:OA_W1 + 128], rh,
                   start=True, stop=False)
                mm(psA[:, co:co + N], p22[:, O22_W1:O22_W1 + 128], rl,
                   start=False, stop=True)
                mm(psB[:, co:co + N], pA[:, OA_W1 + 128:OA_W1 + 234], rh,
                   start=True, stop=False)
                mm(psB[:, co:co + N], p22[:, O22_W1 + 128:O22_W1 + 234], rl,
                   start=False, stop=True)
            a1 = pw.tile([128, 1024], BF16, name=f's1a{j}', tag='s1a')
            b1 = pw.tile([106, 1024], BF16, name=f's1b{j}', tag='s1b')
            nc.scalar.activation(a1[:], psA[:], AF.Lrelu,
                                 bias=bF[0:128, 0:1], alpha=SLOPE)
            nc.scalar.activation(b1[:], psB[:], AF.Lrelu,
                                 bias=bF[0:106, 1:2], alpha=SLOPE)

            psC = pp.tile([40, 1024], F32, name=f'psC{j}', tag='conv')
            for i in range(2):
                co = i * 512
                mm(psC[:, co:co + N], pA[:, OA_C2K1:OA_C2K1 + 40],
                   a1[:, co:co + N], start=True, stop=False)
                mm(psC[:, co:co + N], pS[:, OPS_C2K2:OPS_C2K2 + 40],
                   b1[:, co:co + N], start=False, stop=True)
            nc.scalar.activation(xsml[j][0:40, :], psC[:], AF.Lrelu,
                                 bias=bF[0:40, 2:3], alpha=SLOPE)

        # ---- l-branch transposes (after conv so conv never waits onat) ----
        def l_pair(j):
            lt = pp.tile([3, 1024], F32, name=f'lt{j}', tag='conv')
            for i in range(2):
                for c in range(4):
                    nc.tensor.transpose(
                        lt[0:3, i * 512 + c * NCH: i * 512 + (c + 1) * NCH],
                        lm[2 * j + i][:, c * 3:(c + 1) * 3],
                        bF[0:NCH, 6:6 + NCH])
            nc.scalar.activation(xsml[j][64:67, :], lt[:], AF.Lrelu,
                                 alpha=SLOPE)

        hsb = [[None, None] for _ in range(BL)]

        def h_pair(j):
            for i in range(2):
                b = 2 * j + i
                for half in range(2):
                    ph = pp.tile([NCH, 344], F32, name=f'ph{b}_{half}', tag='h')
                    for c2 in range(2):
                        c = half * 2 + c2
                        mm(ph[:, c2 * 172:(c2 + 1) * 172],
                           xsml[j][0:67, i * 512 + c * NCH: i * 512 + (c + 1) * NCH],
                           pA[0:67, OA_WALL:OA_WALL + 172],
                           start=True, stop=True)
                    t = cp.tile([NCH, 344], BF16, name=f'hsb{b}_{half}',
                                tag=f'hsb{b}_{half}')
                    nc.vector.tensor_copy(t[:], ph[:])
                    hsb[b][half] = t

        xg = [None] * 4

        def agg_pair(j):
            pg = pp.tile([107, N], F32, name=f'pg{j}', tag='agg')
            mm(pg[:], zw[0:1, 0:107], zw[0:1, 0:N], start=True, stop=False)
            mm(pg[0:43, :], pA[0:67, OA_WROOT:OA_WROOT + 43],
               xsml[j][0:67, 0:N], start=False, stop=False)
            mm(pg[64:107, :], pA[0:67, OA_WROOT:OA_WROOT + 43],
               xsml[j][0:67, 512:512 + N], start=False, stop=False,
               tile_position=(0, 64))
            for r in range(R):
                for c in range(4):
                    att = attr[r][:, c * N:(c + 1) * N]
                    mm(pg[0:43, :],
                       hsb[2 * j][c // 2][:, (c % 2) * 172 + r * 43:
                                          (c % 2) * 172 + (r + 1) * 43],
                       att, start=False, stop=False)
                    mm(pg[64:107, :],
                       hsb[2 * j + 1][c // 2][:, (c % 2) * 172 + r * 43:
                                              (c % 2) * 172 + (r + 1) * 43],
                       att, start=False, stop=(r == 3 and c == 3),
                       tile_position=(0, 64))
            x = cp.tile([107, N], BF16, name=f'xg{j}', tag=f'xg{j}')
            nc.scalar.activation(x[:], pg[:], AF.Lrelu,
                                 bias=bF[0:107, 3:4], alpha=SLOPE)
            xg[j] = x

        zpr = [None] * 4

        def z_pair(j):
            zp = pp.tile([2, N], F32, name=f'zp{j}', tag='agg')
            mm(zp[:], pA[0:107, OA_WZG2:OA_WZG2 + 2], xg[j][:],
               start=True, stop=False)
            mm(zp[0:1, :], pA[0:67, OA_WZPT:OA_WZPT + 1],
               xsml[j][0:67, 0:N], start=False, stop=False)
            mm(zp[:], pA[0:67, OA_WZPT2:OA_WZPT2 + 2],
               xsml[j][0:67, 512:512 + N], start=False, stop=True)
            z = pw.tile([2, N], BF16, name=f'zpr{j}', tag='zpr')
            nc.vector.tensor_copy(z[:], zp[:])
            zpr[j] = z

        ztsb = cp.tile([NCH, 32], BF16, name='ztsb', tag='ztsb')

        def zt_pair(j):
            ztq = pp.tile([NCH, 8], BF16, name=f'ztq{j}', tag='agg')
            for c in range(4):
                nc.tensor.transpose(
                    ztq[:, c * 2:(c + 1) * 2],
                    zpr[j][0:2, c * NCH:(c + 1) * NCH],
                    pS[0:2, OPS_ID8:OPS_ID8 + 2])
            nc.vector.tensor_copy(
                ztsb[:].rearrange("p (c b) -> p c b", c=4)[:, :, 2 * j:2 * j + 2],
                ztq[:].rearrange("p (c b) -> p c b", c=4))

        l_pair(0)
        l_pair(1)
        h_pair(0)
        l_pair(2)
        h_pair(1)
        l_pair(3)
        h_pair(2)
        h_pair(3)
        agg_pair(0)
        agg_pair(1)
        z_pair(0)
        agg_pair(2)
        z_pair(1)
        zt_pair(0)
        agg_pair(3)
        z_pair(2)
        zt_pair(1)
        z_pair(3)
        zt_pair(2)
        zt_pair(3)

        # ---- actor head ----
        pg1 = pp.tile([H, BL], F32, name='pg1', tag='agg')
        for c in range(8):
            rhs = (ztsb[:, c * 8:(c + 1) * 8] if c < 4 else
                   wT[0:NCH, OT_ATS + (c - 4) * 8: OT_ATS + (c - 3) * 8])
            mm(pg1[:], wT[0:NCH, OT_W1C + c * H: OT_W1C + (c + 1) * H], rhs,
               start=(c == 0), stop=(c == 7))
        g1 = cp.tile([H, BL], BF16, name='g1', tag='g1')
        nc.scalar.activation(g1[:], pg1[:], AF.Relu, bias=bF[0:128, 4:5])
        pg2 = pp.tile([H, BL], F32, name='pg2', tag='agg')
        mm(pg2[:], pA[:, OA_AW2:OA_AW2 + 128], g1[:], start=True, stop=True)
        g2 = cp.tile([H, BL], BF16, name='g2', tag='g2')
        nc.scalar.activation(g2[:], pg2[:], AF.Relu, bias=bF[0:128, 5:6])

        po = pp.tile([BL, P + 1], F32, name='po', tag='agg')
        mm(po[:], g2[:], wT[:, OT_AW3:OT_AW3 + 501], start=True, stop=False)
        mm(po[:], pA[0:1, OA_ONES:OA_ONES + 8],
           pA[0:1, OA_B3R:OA_B3R + 501], start=False, stop=True)

        mx = pw.tile([BL, 1], F32, name='mx', tag='mx')
        nc.vector.tensor_reduce(mx[:], po[:], axis=AX.X, op=ALU.max)
        sh = pw.tile([BL, P + 1], F32, name='sh', tag='sh')
        nc.vector.tensor_scalar(sh[:], po[:], mx[:, 0:1], None,
                                op0=ALU.subtract)
        ex = pw.tile([BL, P + 1], F32, name='ex', tag='ex')
        sm = pw.tile([BL, 1], F32, name='sm', tag='sm')
        nc.scalar.activation(ex[:], sh[:], AF.Exp, accum_out=sm[:, 0:1])
        rc = pw.tile([BL, 1], F32, name='rc', tag='rc')
        nc.vector.reciprocal(rc[:], sm[:])
        res = pw.tile([BL, P + 1], F32, name='res', tag='res')
        nc.vector.tensor_scalar(res[:], ex[:], rc[:, 0:1], None, op0=ALU.mult)
        nc.sync.dma_start(out=out_d[:], in_=res[:])

    nc.compile()
    return nc


def _get_nc():
    if 'nc' not in _CACHE:
        _CACHE['nc'] = _build_nc()
    return _CACHE['nc']


# ============================ entry point ============================

def _shard_inputs(inputs):
    folded = _host_fold(inputs)
    obs = np.asarray(inputs['observation'], np.float32)
    action = np.asarray(inputs['action'], np.float32)
    obs_t = np.ascontiguousarray(obs.transpose(0, 1, 3, 2)).reshape(B, 150, N)

    in_maps = []
    for i in range(NCORES):
        bs = slice(i * BL, (i + 1) * BL)
        ot = obs_t[bs]
        rh = ot[:, 0:128, :].transpose(1, 0, 2).reshape(128, BL * N)
        rl = ot[:, 128:150, :].transpose(1, 0, 2).reshape(22, BL * N)
        onat = (obs[bs].reshape(BL, C0, 4, NCH, T)
                .transpose(3, 0, 2, 1, 4).reshape(NCH, 4800))
        ats = (action[bs, 1:].reshape(BL, 4, NCH)
               .transpose(2, 1, 0).reshape(NCH, 32))
        wt = folded['wt_const'].copy()
        wt[0:NCH, OT_ATS:OT_ATS + 32] = ats.astype(BF)
        in_maps.append({
            'packA': np.concatenate([folded['pa_const'], rh.astype(BF)], axis=1),
            'pack22': np.concatenate([folded['p22_const'], rl.astype(BF)], axis=1),
            'pdsmall': folded['ps_const'],
            'onat0': np.ascontiguousarray(onat[:, 0:2400]).astype(BF),
            'onat1': np.ascontiguousarray(onat[:, 2400:4800]).astype(BF),
            'attr0': folded['attr'][0], 'attr1': folded['attr'][1],
            'attr2': folded['attr'][2], 'attr3': folded['attr'][3],
            'wtail': wt, 'biasF': folded['biasf'],
        })
    return in_maps


def kernel(**inputs) -> np.ndarray:
    from concourse.bass_utils import run_bass_kernel_spmd

    in_maps = _shard_inputs(inputs)
    nc = _get_nc()
    res = run_bass_kernel_spmd(nc, in_maps, list(range(NCORES)))
    return np.concatenate([np.asarray(r['out'], np.float32)
                           for r in res.results], axis=0)


# revision 7
# speedup vs baseline: 1.0116x; 1.0116x over previous
"""Trainium2 Bass kernel for nn_CustomGPM (multi-scale temporal CNN + RGCN + actor head).

v3: bf16 datapath, DMA spread over all 5 engine queues with per-relation
adjacency chunks, DMA-independent PE warmup + scalar table priming,
pair-packed conv psums/activations, col-tiled RGCN aggregation, and a
DMA-free z-transpose tail.

Layout per core (BL=8 batch elems, 4 pairs):
  xsml[j] [67, 1024] bf16: rows 0:20 s-conv2, 20:40 m-conv2, 40:64 zero,
    64:67 l(max_t); cols b0 0:500, b1 512:1012 (bank-aligned regions)
  hsb[b][i] [125, 344] bf16: node-on-partition H = x^T W_rel, chunks 2i,2i+1
  agg psum [107, 500]: rows 0:43 b0 feats, 64:107 b1 (col-tiled matmuls)
"""

import numpy as np
import ml_dtypes

BF = ml_dtypes.bfloat16

# ---------------- problem constants (hardcoded per spec) ----------------
B = 64
NCORES = 8
BL = B // NCORES          # 8 per core, 4 pairs
C0, N, T, R, P, H = 3, 500, 50, 4, 500, 128
CF = 20
F = 2 * CF + C0           # 43
NCH = 125
TS1, TM1 = 48, 30
SLOPE = 0.01
EPS = 1e-5

# packA (bf16, 128 partitions) column offsets
OA_W1 = 0                 # [150 -> 128+22 split, 234] merged conv1 band
OA_C2K1 = 234             # [128, 40]
OA_AW2 = OA_C2K1 + 40     # [128, 128]
OA_WALL = OA_AW2 + 128    # [67, 172]
OA_WROOT = OA_WALL + 172  # [67, 43]
OA_WZPT = OA_WROOT + 43   # [67, 1]
OA_WZPT2 = OA_WZPT + 1    # [67, 2]
OA_WZG2 = OA_WZPT2 + 2    # [107, 2]
OA_B3R = OA_WZG2 + 2      # [1, 501]
OA_ONES = OA_B3R + 501    # [1, 8]
OA_RH = OA_ONES + 8       # [128, 4000] obs rows 0:128, cols b*500+n
CA = OA_RH + 4000

# pack22 (bf16, 22 partitions)
O22_W1 = 0                # [22, 234] conv1 band rows 128:150
O22_RL = 234              # [22, 4000] obs rows 128:150
C22 = O22_RL + 4000

# pdsmall (bf16, 106 partitions): conv2 K-tile2 + identity
OPS_C2K2 = 0              # [106, 40]
OPS_ID8 = 40              # [8, 8] identity
CPS = 48

# wtail (bf16, 128 partitions): tail-phase weights
OT_AW3 = 0                # [128, 501]
OT_W1C = 501              # [125, 1024] fc1 chunks
OT_ATS = OT_W1C + 1024    # [125, 32] action^T (c, b)
CT = OT_ATS + 32

_CACHE = {}


# ======================= host-side parameter folding =======================

def _bn_fold(p):
    g, b, m, v = np.asarray(p, np.float64)
    s = g / np.sqrt(v + EPS)
    return s, b - m * s


def _conv_band_lhsT(w, bias, bn, t_out):
    w = np.asarray(w, np.float64)[:, :, 0, :]
    co, ci, k = w.shape
    s, t_ = _bn_fold(bn)
    w_eff = w * s[:, None, None]
    b_eff = s * np.asarray(bias, np.float64) + t_
    band = np.zeros((co, t_out, ci, T), np.float64)
    for t in range(t_out):
        band[:, t, :, t:t + k] = w_eff
    lhsT = band.reshape(co * t_out, ci * T).T.copy()
    return lhsT, np.repeat(b_eff, t_out)


def _conv2_fold(w, b, bn):
    w = np.asarray(w, np.float64)[:, :, 0, :]
    s, t_ = _bn_fold(bn)
    w_eff = (w * s[:, None, None]).reshape(CF, -1)
    b_eff = s * np.asarray(b, np.float64) + t_
    return w_eff.T.copy(), b_eff


def _pad67(a):
    """[43, X] -> [67, X]: rows 0:40 = a[0:40], 64:67 = a[40:43]."""
    out = np.zeros((67,) + a.shape[1:], np.float64)
    out[0:40] = a[0:40]
    out[64:67] = a[40:43]
    return out


def _host_fold(inp):
    ws1, bs1 = _conv_band_lhsT(inp['sc1_w'], inp['sc1_b'], inp['sbn1'], TS1)
    wm1, bm1 = _conv_band_lhsT(inp['mc1_w'], inp['mc1_b'], inp['mbn1'], TM1)
    w1all = np.concatenate([ws1, wm1], axis=1)              # [150, 234]
    bias_a = bs1[0:128]
    bias_b = np.concatenate([bs1[128:144], bm1])            # [106]

    ws2, bs2 = _conv2_fold(inp['sc2_w'], inp['sc2_b'], inp['sbn2'])  # [144,20]
    wm2, bm2 = _conv2_fold(inp['mc2_w'], inp['mc2_b'], inp['mbn2'])  # [90,20]
    c2k1 = np.zeros((128, 40), np.float64)
    c2k1[:, 0:20] = ws2[0:128]
    c2k2 = np.zeros((106, 40), np.float64)
    c2k2[0:16, 0:20] = ws2[128:144]
    c2k2[16:106, 20:40] = wm2
    bias_c = np.concatenate([bs2, bm2])                     # [40]

    sg, tg = _bn_fold(inp['gbn'])
    w_all = np.concatenate(
        [np.asarray(inp['gw_rel'], np.float64)[r] * sg[None, :]
         for r in range(R)], axis=1)                        # [43, 172]
    w_root = np.asarray(inp['gw_root'], np.float64) * sg[None, :]
    gb_eff = np.asarray(inp['g_b'], np.float64) * sg + tg
    wallt = _pad67(w_all)
    wroott = _pad67(w_root)

    src = np.asarray(inp['edge_index'][0]).astype(np.int64)
    dst = np.asarray(inp['edge_index'][1]).astype(np.int64)
    etype = np.asarray(inp['edge_type']).astype(np.int64)
    a_t = np.zeros((R, N, N), np.float64)                   # [r, src, dst]
    for r in range(R):
        sel = etype == r
        cnt = np.zeros((N, N), np.float64)
        np.add.at(cnt, (dst[sel], src[sel]), 1.0)
        deg = cnt.sum(axis=1)
        a_t[r] = (cnt / np.maximum(deg, 1.0)[:, None]).T
    # per relation: [125, (c, n)] with src chunked on partitions
    attr = [np.ascontiguousarray(
        a_t[r].reshape(4, NCH, N).transpose(1, 0, 2).reshape(NCH, 4 * N)
    ).astype(BF) for r in range(R)]

    a_cw = np.asarray(inp['a_cw'], np.float64)
    a_cb = float(np.asarray(inp['a_cb'], np.float64)[0])
    a_w1 = np.asarray(inp['a_w1'], np.float64)
    sel_nodes = np.asarray(inp['nodes_to_select']).astype(np.int64)
    w_z = a_cw[1:1 + 2 * F]
    wzpt = _pad67(w_z[0:F].reshape(F, 1))
    wzpt2 = np.zeros((67, 2), np.float64)
    wzpt2[:, 1:2] = wzpt
    wzg2 = np.zeros((107, 2), np.float64)
    wzg2[0:43, 0] = w_z[F:]
    wzg2[64:107, 1] = w_z[F:]
    col_g = np.zeros(107, np.float64)
    col_g[0:43] = gb_eff
    col_g[64:107] = gb_eff

    w1z = np.zeros((N, H), np.float64)
    np.add.at(w1z, sel_nodes, a_w1[1:])
    w1a = a_cw[0] * a_w1[1:]
    b1_eff = np.asarray(inp['a_b1'], np.float64) + a_cb * a_w1[1:].sum(axis=0)
    w1cat = np.concatenate([w1z, w1a], axis=0)              # [1000, 128]
    w1c = w1cat.reshape(8, NCH, H).transpose(1, 0, 2).reshape(NCH, 8 * H)

    biasf = np.zeros((128, 6 + NCH), np.float32)
    biasf[0:128, 0] = bias_a
    biasf[0:106, 1] = bias_b
    biasf[0:40, 2] = bias_c
    biasf[0:107, 3] = col_g
    biasf[0:128, 4] = b1_eff
    biasf[0:128, 5] = np.asarray(inp['a_b2'], np.float64)
    biasf[0:NCH, 6:6 + NCH] = np.eye(NCH)                   # f32 transpose id

    pa = np.zeros((128, OA_RH), np.float64)
    pa[:, OA_W1:OA_W1 + 234] = w1all[0:128]
    pa[:, OA_C2K1:OA_C2K1 + 40] = c2k1
    pa[:, OA_AW2:OA_AW2 + 128] = np.asarray(inp['a_w2'], np.float64)
    pa[0:67, OA_WALL:OA_WALL + 172] = wallt
    pa[0:67, OA_WROOT:OA_WROOT + 43] = wroott
    pa[0:67, OA_WZPT:OA_WZPT + 1] = wzpt
    pa[0:67, OA_WZPT2:OA_WZPT2 + 2] = wzpt2
    pa[0:107, OA_WZG2:OA_WZG2 + 2] = wzg2
    pa[0:1, OA_B3R:OA_B3R + 501] = np.asarray(inp['a_b3'], np.float64)
    pa[0:1, OA_ONES:OA_ONES + 8] = 1.0

    ps = np.zeros((106, CPS), np.float64)
    ps[0:106, OPS_C2K2:OPS_C2K2 + 40] = c2k2
    ps[0:8, OPS_ID8:OPS_ID8 + 8] = np.eye(8)

    wt = np.zeros((128, CT), np.float64)
    wt[:, OT_AW3:OT_AW3 + 501] = np.asarray(inp['a_w3'], np.float64)
    wt[0:NCH, OT_W1C:OT_W1C + 1024] = w1c

    return {
        'pa_const': pa.astype(BF), 'p22_const': w1all[128:150].astype(BF),
        'ps_const': ps.astype(BF), 'wt_const': wt.astype(BF),
        'attr': attr, 'biasf': biasf,
    }


# ============================ device kernel ============================

def _build_nc():
    import concourse.bacc as bacc
    import concourse.tile as tile
    import concourse.mybir as mybir
    from contextlib import ExitStack

    F32 = mybir.dt.float32
    BF16 = mybir.dt.bfloat16
    AF = mybir.ActivationFunctionType
    ALU = mybir.AluOpType
    AX = mybir.AxisListType

    nc = bacc.Bacc("TRN2", target_bir_lowering=False, debug=False)

    packA_d = nc.dram_tensor('packA', [128, CA], BF16, kind="ExternalInput").ap()
    pack22_d = nc.dram_tensor('pack22', [22, C22], BF16, kind="ExternalInput").ap()
    pds_d = nc.dram_tensor('pdsmall', [106, CPS], BF16, kind="ExternalInput").ap()
    onat0_d = nc.dram_tensor('onat0', [NCH, 2400], BF16, kind="ExternalInput").ap()
    onat1_d = nc.dram_tensor('onat1', [NCH, 2400], BF16, kind="ExternalInput").ap()
    attr_d = [nc.dram_tensor(f'attr{r}', [NCH, 4 * N], BF16,
                             kind="ExternalInput").ap() for r in range(R)]
    wtail_d = nc.dram_tensor('wtail', [128, CT], BF16, kind="ExternalInput").ap()
    biasF_d = nc.dram_tensor('biasF', [128, 6 + NCH], F32,
                             kind="ExternalInput").ap()
    out_d = nc.dram_tensor('out', [BL, P + 1], F32, kind="ExternalOutput").ap()

    mm = nc.tensor.matmul

    with tile.TileContext(nc) as tc, ExitStack() as ctx:
        cp = ctx.enter_context(tc.tile_pool(name="const", bufs=1))
        pw = ctx.enter_context(tc.tile_pool(name="work", bufs=2))
        pp = ctx.enter_context(tc.tile_pool(name="pp", bufs=2, space="PSUM"))

        zw = cp.tile([128, 512], BF16, name='zw', tag='zw')
        nc.gpsimd.memset(zw[:], 0)

        pA = cp.tile([128, CA], BF16, name='pA', tag='pA')
        p22 = cp.tile([22, C22], BF16, name='p22', tag='p22')
        pS = cp.tile([106, CPS], BF16, name='pS', tag='pS')
        on0 = cp.tile([NCH, 2400], BF16, name='on0', tag='on0')
        on1 = cp.tile([NCH, 2400], BF16, name='on1', tag='on1')
        attr = [cp.tile([NCH, 4 * N], BF16, name=f'attr{r}', tag=f'attr{r}')
                for r in range(R)]
        wT = cp.tile([128, CT], BF16, name='wT', tag='wT')
        bF = cp.tile([128, 6 + NCH], F32, name='bF', tag='bF')

        # DMA plan: spread across all five engine queues
        nc.sync.dma_start(out=pA[:], in_=packA_d[:])
        nc.sync.dma_start(out=attr[0][:], in_=attr_d[0][:])
        nc.sync.dma_start(out=attr[2][:], in_=attr_d[2][:])
        nc.scalar.dma_start(out=p22[:], in_=pack22_d[:])
        nc.scalar.dma_start(out=pS[:], in_=pds_d[:])
        nc.scalar.dma_start(out=bF[:], in_=biasF_d[:])
        nc.scalar.dma_start(out=attr[1][:], in_=attr_d[1][:])
        nc.scalar.dma_start(out=attr[3][:], in_=attr_d[3][:])
        nc.scalar.dma_start(out=wT[:], in_=wtail_d[:])
        nc.gpsimd.dma_start(out=on0[:], in_=onat0_d[:])
        nc.gpsimd.dma_start(out=on1[:], in_=onat1_d[:])

        xsml = [cp.tile([67, 1024], BF16, name=f'xsml{j}', tag=f'xsml{j}')
                for j in range(4)]
        for j in range(4):
            nc.gpsimd.memset(xsml[j][32:64, :], 0)

        # scalar priming: pull the LEAKY_RELU table load to kernel start
        prim = cp.tile([1, 8], BF16, name='prim', tag='prim')
        nc.scalar.activation(prim[:], zw[0:1, 0:8], AF.Lrelu, alpha=SLOPE)

        # ---- PE warmup on zeros (HAM to K=8/8 while DMAs land) ----
        for w in range(18):
            pwm = pp.tile([128, 512], F32, name=f'pwm{w}', tag='conv')
            mm(pwm[:], zw[:, 0:128], zw[:], start=True, stop=True)

        # ---- l-branch max over t (DVE, early) ----
        lm = []
        for b in range(BL):
            t = cp.tile([NCH, 12], F32, name=f'lm{b}', tag=f'lm{b}')
            src = on0 if b < 4 else on1
            nc.vector.tensor_reduce(
                t[:],
                src[:, (b % 4) * 600:(b % 4 + 1) * 600].rearrange(
                    "p (c k t) -> p c k t", c=4, k=C0),
                axis=AX.X, op=ALU.max)
            lm.append(t)

        # ---- conv pairs (PE + ACT; no l-branch deps here) ----
        for j in range(4):
            psA = pp.tile([128, 1024], F32, name=f'psA{j}', tag='conv')
            psB = pp.tile([106, 1024], F32, name=f'psB{j}', tag='conv')
            for i in range(2):
                b = 2 * j + i
                rh = pA[:, OA_RH + b * N: OA_RH + (b + 1) * N]
                rl = p22[:, O22_RL + b * N: O22_RL + (b + 1) * N]
                co = i * 512
                mm(psA[:, co:co + N], pA[:, OA_W1:OA_W1 + 128], rh,
                   start=True, stop=False)
                mm(psA[:, co:co + N], p22[:, O22_W1:O22_W1 + 128], rl,
                   start=False, stop=True)
                mm(psB[:, co:co + N], pA[:, OA_W1 + 128:OA_W1 + 234], rh,
                   start=True, stop=False)
                mm(psB[:, co:co + N], p22[:, O22_W1 + 128:O22_W1 + 234], rl,
                   start=False, stop=True)
            a1 = pw.tile([128, 1024], BF16, name=f's1a{j}', tag='s1a')
            b1 = pw.tile([106, 1024], BF16, name=f's1b{j}', tag='s1b')
            nc.scalar.activation(a1[:], psA[:], AF.Lrelu,
                                 bias=bF[0:128, 0:1], alpha=SLOPE)
            nc.scalar.activation(b1[:], psB[:], AF.Lrelu,
                                 bias=bF[0:106, 1:2], alpha=SLOPE)

            psC = pp.tile([40, 1024], F32, name=f'psC{j}', tag='conv')
            for i in range(2):
                co = i * 512
                mm(psC[:, co:co + N], pA[:, OA_C2K1:OA_C2K1 + 40],
                   a1[:, co:co + N], start=True, stop=False)
                mm(psC[:, co:co + N], pS[:, OPS_C2K2:OPS_C2K2 + 40],
                   b1[:, co:co + N], start=False, stop=True)
            nc.scalar.activation(xsml[j][0:40, :], psC[:], AF.Lrelu,
                                 bias=bF[0:40, 2:3], alpha=SLOPE)

        # ---- l-branch transposes (after conv so conv never waits onat) ----
        def l_pair(j):
            lt = pp.tile([3, 1024], F32, name=f'lt{j}', tag='conv')
            for i in range(2):
                for c in range(4):
                    nc.tensor.transpose(
                        lt[0:3, i * 512 + c * NCH: i * 512 + (c + 1) * NCH],
                        lm[2 * j + i][:, c * 3:(c + 1) * 3],
                        bF[0:NCH, 6:6 + NCH])
            nc.scalar.activation(xsml[j][64:67, :], lt[:], AF.Lrelu,
                                 alpha=SLOPE)

        hsb = [[None, None] for _ in range(BL)]

        def h_pair(j):
            for i in range(2):
                b = 2 * j + i
                for half in range(2):
                    ph = pp.tile([NCH, 344], F32, name=f'ph{b}_{half}', tag='h')
                    for c2 in range(2):
                        c = half * 2 + c2
                        mm(ph[:, c2 * 172:(c2 + 1) * 172],
                           xsml[j][0:67, i * 512 + c * NCH: i * 512 + (c + 1) * NCH],
                           pA[0:67, OA_WALL:OA_WALL + 172],
                           start=True, stop=True)
                    t = cp.tile([NCH, 344], BF16, name=f'hsb{b}_{half}',
                                tag=f'hsb{b}_{half}')
                    nc.vector.tensor_copy(t[:], ph[:])
                    hsb[b][half] = t

        xg = [None] * 4

        def agg_pair(j):
            pg = pp.tile([107, N], F32, name=f'pg{j}', tag='agg')
            mm(pg[:], zw[0:1, 0:107], zw[0:1, 0:N], start=True, stop=False)
            mm(pg[0:43, :], pA[0:67, OA_WROOT:OA_WROOT + 43],
               xsml[j][0:67, 0:N], start=False, stop=False)
            mm(pg[64:107, :], pA[0:67, OA_WROOT:OA_WROOT + 43],
               xsml[j][0:67, 512:512 + N], start=False, stop=False,
               tile_position=(0, 64))
            for r in (1, 0, 3, 2):
                for c in range(4):
                    att = attr[r][:, c * N:(c + 1) * N]
                    mm(pg[0:43, :],
                       hsb[2 * j][c // 2][:, (c % 2) * 172 + r * 43:
                                          (c % 2) * 172 + (r + 1) * 43],
                       att, start=False, stop=False)
                    mm(pg[64:107, :],
                       hsb[2 * j + 1][c // 2][:, (c % 2) * 172 + r * 43:
                                              (c % 2) * 172 + (r + 1) * 43],
                       att, start=False, stop=(r == 2 and c == 3),
                       tile_position=(0, 64))
            x = cp.tile([107, N], BF16, name=f'xg{j}', tag=f'xg{j}')
            nc.scalar.activation(x[:], pg[:], AF.Lrelu,
                                 bias=bF[0:107, 3:4], alpha=SLOPE)
            xg[j] = x

        zpr = [None] * 4

        def z_pair(j):
            zp = pp.tile([2, N], F32, name=f'zp{j}', tag='agg')
            mm(zp[:], pA[0:107, OA_WZG2:OA_WZG2 + 2], xg[j][:],
               start=True, stop=False)
            mm(zp[0:1, :], pA[0:67, OA_WZPT:OA_WZPT + 1],
               xsml[j][0:67, 0:N], start=False, stop=False)
            mm(zp[:], pA[0:67, OA_WZPT2:OA_WZPT2 + 2],
               xsml[j][0:67, 512:512 + N], start=False, stop=True)
            z = pw.tile([2, N], BF16, name=f'zpr{j}', tag='zpr')
            nc.vector.tensor_copy(z[:], zp[:])
            zpr[j] = z

        ztsb = cp.tile([NCH, 32], BF16, name='ztsb', tag='ztsb')

        def zt_pair(j):
            ztq = pp.tile([NCH, 8], BF16, name=f'ztq{j}', tag='agg')
            for c in range(4):
                nc.tensor.transpose(
                    ztq[:, c * 2:(c + 1) * 2],
                    zpr[j][0:2, c * NCH:(c + 1) * NCH],
                    pS[0:2, OPS_ID8:OPS_ID8 + 2])
            nc.vector.tensor_copy(
                ztsb[:].rearrange("p (c b) -> p c b", c=4)[:, :, 2 * j:2 * j + 2],
                ztq[:].rearrange("p (c b) -> p c b", c=4))

        l_pair(0)
        l_pair(1)
        h_pair(0)
        l_pair(2)
        h_pair(1)
        l_pair(3)
        h_pair(2)
        h_pair(3)
        agg_pair(0)
        agg_pair(1)
        z_pair(0)
        agg_pair(2)
        z_pair(1)
        zt_pair(0)
        agg_pair(3)
        z_pair(2)
        zt_pair(1)
        z_pair(3)
        zt_pair(2)
        zt_pair(3)

        # ---- actor head ----
        pg1 = pp.tile([H, BL], F32, name='pg1', tag='agg')
        for c in range(8):
            rhs = (ztsb[:, c * 8:(c + 1) * 8] if c < 4 else
                   wT[0:NCH, OT_ATS + (c - 4) * 8: OT_ATS + (c - 3) * 8])
            mm(pg1[:], wT[0:NCH, OT_W1C + c * H: OT_W1C + (c + 1) * H], rhs,
               start=(c == 0), stop=(c == 7))
        g1 = cp.tile([H, BL], BF16, name='g1', tag='g1')
        nc.scalar.activation(g1[:], pg1[:], AF.Relu, bias=bF[0:128, 4:5])
        pg2 = pp.tile([H, BL], F32, name='pg2', tag='agg')
        mm(pg2[:], pA[:, OA_AW2:OA_AW2 + 128], g1[:], start=True, stop=True)
        g2 = cp.tile([H, BL], BF16, name='g2', tag='g2')
        nc.scalar.activation(g2[:], pg2[:], AF.Relu, bias=bF[0:128, 5:6])

        po = pp.tile([BL, P + 1], F32, name='po', tag='agg')
        mm(po[:], g2[:], wT[:, OT_AW3:OT_AW3 + 501], start=True, stop=False)
        mm(po[:], pA[0:1, OA_ONES:OA_ONES + 8],
           pA[0:1, OA_B3R:OA_B3R + 501], start=False, stop=True)

        mx = pw.tile([BL, 1], F32, name='mx', tag='mx')
        nc.vector.tensor_reduce(mx[:], po[:], axis=AX.X, op=ALU.max)
        sh = pw.tile([BL, P + 1], F32, name='sh', tag='sh')
        nc.vector.tensor_scalar(sh[:], po[:], mx[:, 0:1], None,
                                op0=ALU.subtract)
        ex = pw.tile([BL, P + 1], F32, name='ex', tag='ex')
        sm = pw.tile([BL, 1], F32, name='sm', tag='sm')
        nc.scalar.activation(ex[:], sh[:], AF.Exp, accum_out=sm[:, 0:1])
        rc = pw.tile([BL, 1], F32, name='rc', tag='rc')
        nc.vector.reciprocal(rc[:], sm[:])
        res = pw.tile([BL, P + 1], F32, name='res', tag='res')
        nc.vector.tensor_scalar(res[:], ex[:], rc[:, 0:1], None, op0=ALU.mult)
        nc.sync.dma_start(out=out_d[:], in_=res[:])

    nc.compile()
    return nc


def _get_nc():
    if 'nc' not in _CACHE:
        _CACHE['nc'] = _build_nc()
    return _CACHE['nc']


# ============================ entry point ============================

def _shard_inputs(inputs):
    folded = _host_fold(inputs)
    obs = np.asarray(inputs['observation'], np.float32)
    action = np.asarray(inputs['action'], np.float32)
    obs_t = np.ascontiguousarray(obs.transpose(0, 1, 3, 2)).reshape(B, 150, N)

    in_maps = []
    for i in range(NCORES):
        bs = slice(i * BL, (i + 1) * BL)
        ot = obs_t[bs]
        rh = ot[:, 0:128, :].transpose(1, 0, 2).reshape(128, BL * N)
        rl = ot[:, 128:150, :].transpose(1, 0, 2).reshape(22, BL * N)
        onat = (obs[bs].reshape(BL, C0, 4, NCH, T)
                .transpose(3, 0, 2, 1, 4).reshape(NCH, 4800))
        ats = (action[bs, 1:].reshape(BL, 4, NCH)
               .transpose(2, 1, 0).reshape(NCH, 32))
        wt = folded['wt_const'].copy()
        wt[0:NCH, OT_ATS:OT_ATS + 32] = ats.astype(BF)
        in_maps.append({
            'packA': np.concatenate([folded['pa_const'], rh.astype(BF)], axis=1),
            'pack22': np.concatenate([folded['p22_const'], rl.astype(BF)], axis=1),
            'pdsmall': folded['ps_const'],
            'onat0': np.ascontiguousarray(onat[:, 0:2400]).astype(BF),
            'onat1': np.ascontiguousarray(onat[:, 2400:4800]).astype(BF),
            'attr0': folded['attr'][0], 'attr1': folded['attr'][1],
            'attr2': folded['attr'][2], 'attr3': folded['attr'][3],
            'wtail': wt, 'biasF': folded['biasf'],
        })
    return in_maps


def kernel(**inputs) -> np.ndarray:
    from concourse.bass_utils import run_bass_kernel_spmd

    in_maps = _shard_inputs(inputs)
    nc = _get_nc()
    res = run_bass_kernel_spmd(nc, in_maps, list(range(NCORES)))
    return np.concatenate([np.asarray(r['out'], np.float32)
                           for r in res.results], axis=0)


# revision 8
# speedup vs baseline: 1.0166x; 1.0050x over previous
"""Trainium2 Bass kernel for nn_CustomGPM (multi-scale temporal CNN + RGCN + actor head).

v3: bf16 datapath, DMA spread over all 5 engine queues with per-relation
adjacency chunks, DMA-independent PE warmup + scalar table priming,
pair-packed conv psums/activations, col-tiled RGCN aggregation, and a
DMA-free z-transpose tail.

Layout per core (BL=8 batch elems, 4 pairs):
  xsml[j] [67, 1024] bf16: rows 0:20 s-conv2, 20:40 m-conv2, 40:64 zero,
    64:67 l(max_t); cols b0 0:500, b1 512:1012 (bank-aligned regions)
  hsb[b][i] [125, 344] bf16: node-on-partition H = x^T W_rel, chunks 2i,2i+1
  agg psum [107, 500]: rows 0:43 b0 feats, 64:107 b1 (col-tiled matmuls)
"""

import numpy as np
import ml_dtypes

BF = ml_dtypes.bfloat16

# ---------------- problem constants (hardcoded per spec) ----------------
B = 64
NCORES = 8
BL = B // NCORES          # 8 per core, 4 pairs
C0, N, T, R, P, H = 3, 500, 50, 4, 500, 128
CF = 20
F = 2 * CF + C0           # 43
NCH = 125
TS1, TM1 = 48, 30
SLOPE = 0.01
EPS = 1e-5

# packA (bf16, 128 partitions) column offsets
OA_W1 = 0                 # [150 -> 128+22 split, 234] merged conv1 band
OA_C2K1 = 234             # [128, 40]
OA_AW2 = OA_C2K1 + 40     # [128, 128]
OA_WALL = OA_AW2 + 128    # [67, 172]
OA_WROOT = OA_WALL + 172  # [67, 43]
OA_WZPT = OA_WROOT + 43   # [67, 1]
OA_WZPT2 = OA_WZPT + 1    # [67, 2]
OA_WZG2 = OA_WZPT2 + 2    # [107, 2]
OA_B3R = OA_WZG2 + 2      # [1, 501]
OA_ONES = OA_B3R + 501    # [1, 8]
OA_RH = OA_ONES + 8       # [128, 4000] obs rows 0:128, cols b*500+n
CA = OA_RH + 4000

# pack22 (bf16, 22 partitions)
O22_W1 = 0                # [22, 234] conv1 band rows 128:150
O22_RL = 234              # [22, 4000] obs rows 128:150
C22 = O22_RL + 4000

# pdsmall (bf16, 106 partitions): conv2 K-tile2 + identity
OPS_C2K2 = 0              # [106, 40]
OPS_ID8 = 40              # [8, 8] identity
CPS = 48

# wtail (bf16, 128 partitions): tail-phase weights
OT_AW3 = 0                # [128, 501]
OT_W1C = 501              # [125, 1024] fc1 chunks
OT_ATS = OT_W1C + 1024    # [125, 32] action^T (c, b)
CT = OT_ATS + 32

_CACHE = {}


# ======================= host-side parameter folding =======================

def _bn_fold(p):
    g, b, m, v = np.asarray(p, np.float64)
    s = g / np.sqrt(v + EPS)
    return s, b - m * s


def _conv_band_lhsT(w, bias, bn, t_out):
    w = np.asarray(w, np.float64)[:, :, 0, :]
    co, ci, k = w.shape
    s, t_ = _bn_fold(bn)
    w_eff = w * s[:, None, None]
    b_eff = s * np.asarray(bias, np.float64) + t_
    band = np.zeros((co, t_out, ci, T), np.float64)
    for t in range(t_out):
        band[:, t, :, t:t + k] = w_eff
    lhsT = band.reshape(co * t_out, ci * T).T.copy()
    return lhsT, np.repeat(b_eff, t_out)


def _conv2_fold(w, b, bn):
    w = np.asarray(w, np.float64)[:, :, 0, :]
    s, t_ = _bn_fold(bn)
    w_eff = (w * s[:, None, None]).reshape(CF, -1)
    b_eff = s * np.asarray(b, np.float64) + t_
    return w_eff.T.copy(), b_eff


def _pad67(a):
    """[43, X] -> [67, X]: rows 0:40 = a[0:40], 64:67 = a[40:43]."""
    out = np.zeros((67,) + a.shape[1:], np.float64)
    out[0:40] = a[0:40]
    out[64:67] = a[40:43]
    return out


def _host_fold(inp):
    ws1, bs1 = _conv_band_lhsT(inp['sc1_w'], inp['sc1_b'], inp['sbn1'], TS1)
    wm1, bm1 = _conv_band_lhsT(inp['mc1_w'], inp['mc1_b'], inp['mbn1'], TM1)
    w1all = np.concatenate([ws1, wm1], axis=1)              # [150, 234]
    bias_a = bs1[0:128]
    bias_b = np.concatenate([bs1[128:144], bm1])            # [106]

    ws2, bs2 = _conv2_fold(inp['sc2_w'], inp['sc2_b'], inp['sbn2'])  # [144,20]
    wm2, bm2 = _conv2_fold(inp['mc2_w'], inp['mc2_b'], inp['mbn2'])  # [90,20]
    c2k1 = np.zeros((128, 40), np.float64)
    c2k1[:, 0:20] = ws2[0:128]
    c2k2 = np.zeros((106, 40), np.float64)
    c2k2[0:16, 0:20] = ws2[128:144]
    c2k2[16:106, 20:40] = wm2
    bias_c = np.concatenate([bs2, bm2])                     # [40]

    sg, tg = _bn_fold(inp['gbn'])
    w_all = np.concatenate(
        [np.asarray(inp['gw_rel'], np.float64)[r] * sg[None, :]
         for r in range(R)], axis=1)                        # [43, 172]
    w_root = np.asarray(inp['gw_root'], np.float64) * sg[None, :]
    gb_eff = np.asarray(inp['g_b'], np.float64) * sg + tg
    wallt = _pad67(w_all)
    wroott = _pad67(w_root)

    src = np.asarray(inp['edge_index'][0]).astype(np.int64)
    dst = np.asarray(inp['edge_index'][1]).astype(np.int64)
    etype = np.asarray(inp['edge_type']).astype(np.int64)
    a_t = np.zeros((R, N, N), np.float64)                   # [r, src, dst]
    for r in range(R):
        sel = etype == r
        cnt = np.zeros((N, N), np.float64)
        np.add.at(cnt, (dst[sel], src[sel]), 1.0)
        deg = cnt.sum(axis=1)
        a_t[r] = (cnt / np.maximum(deg, 1.0)[:, None]).T
    # per relation: [125, (c, n)] with src chunked on partitions
    attr = [np.ascontiguousarray(
        a_t[r].reshape(4, NCH, N).transpose(1, 0, 2).reshape(NCH, 4 * N)
    ).astype(BF) for r in range(R)]

    a_cw = np.asarray(inp['a_cw'], np.float64)
    a_cb = float(np.asarray(inp['a_cb'], np.float64)[0])
    a_w1 = np.asarray(inp['a_w1'], np.float64)
    sel_nodes = np.asarray(inp['nodes_to_select']).astype(np.int64)
    w_z = a_cw[1:1 + 2 * F]
    wzpt = _pad67(w_z[0:F].reshape(F, 1))
    wzpt2 = np.zeros((67, 2), np.float64)
    wzpt2[:, 1:2] = wzpt
    wzg2 = np.zeros((107, 2), np.float64)
    wzg2[0:43, 0] = w_z[F:]
    wzg2[64:107, 1] = w_z[F:]
    col_g = np.zeros(107, np.float64)
    col_g[0:43] = gb_eff
    col_g[64:107] = gb_eff

    w1z = np.zeros((N, H), np.float64)
    np.add.at(w1z, sel_nodes, a_w1[1:])
    w1a = a_cw[0] * a_w1[1:]
    b1_eff = np.asarray(inp['a_b1'], np.float64) + a_cb * a_w1[1:].sum(axis=0)
    w1cat = np.concatenate([w1z, w1a], axis=0)              # [1000, 128]
    w1c = w1cat.reshape(8, NCH, H).transpose(1, 0, 2).reshape(NCH, 8 * H)

    biasf = np.zeros((128, 6 + NCH), np.float32)
    biasf[0:128, 0] = bias_a
    biasf[0:106, 1] = bias_b
    biasf[0:40, 2] = bias_c
    biasf[0:107, 3] = col_g
    biasf[0:128, 4] = b1_eff
    biasf[0:128, 5] = np.asarray(inp['a_b2'], np.float64)
    biasf[0:NCH, 6:6 + NCH] = np.eye(NCH)                   # f32 transpose id

    pa = np.zeros((128, OA_RH), np.float64)
    pa[:, OA_W1:OA_W1 + 234] = w1all[0:128]
    pa[:, OA_C2K1:OA_C2K1 + 40] = c2k1
    pa[:, OA_AW2:OA_AW2 + 128] = np.asarray(inp['a_w2'], np.float64)
    pa[0:67, OA_WALL:OA_WALL + 172] = wallt
    pa[0:67, OA_WROOT:OA_WROOT + 43] = wroott
    pa[0:67, OA_WZPT:OA_WZPT + 1] = wzpt
    pa[0:67, OA_WZPT2:OA_WZPT2 + 2] = wzpt2
    pa[0:107, OA_WZG2:OA_WZG2 + 2] = wzg2
    pa[0:1, OA_B3R:OA_B3R + 501] = np.asarray(inp['a_b3'], np.float64)
    pa[0:1, OA_ONES:OA_ONES + 8] = 1.0

    ps = np.zeros((106, CPS), np.float64)
    ps[0:106, OPS_C2K2:OPS_C2K2 + 40] = c2k2
    ps[0:8, OPS_ID8:OPS_ID8 + 8] = np.eye(8)

    wt = np.zeros((128, CT), np.float64)
    wt[:, OT_AW3:OT_AW3 + 501] = np.asarray(inp['a_w3'], np.float64)
    wt[0:NCH, OT_W1C:OT_W1C + 1024] = w1c

    return {
        'pa_const': pa.astype(BF), 'p22_const': w1all[128:150].astype(BF),
        'ps_const': ps.astype(BF), 'wt_const': wt.astype(BF),
        'attr': attr, 'biasf': biasf,
    }


# ============================ device kernel ============================

def _build_nc():
    import concourse.bacc as bacc
    import concourse.tile as tile
    import concourse.mybir as mybir
    from contextlib import ExitStack

    F32 = mybir.dt.float32
    BF16 = mybir.dt.bfloat16
    AF = mybir.ActivationFunctionType
    ALU = mybir.AluOpType
    AX = mybir.AxisListType

    nc = bacc.Bacc("TRN2", target_bir_lowering=False, debug=False)

    packA_d = nc.dram_tensor('packA', [128, CA], BF16, kind="ExternalInput").ap()
    pack22_d = nc.dram_tensor('pack22', [22, C22], BF16, kind="ExternalInput").ap()
    pds_d = nc.dram_tensor('pdsmall', [106, CPS], BF16, kind="ExternalInput").ap()
    onat0_d = nc.dram_tensor('onat0', [NCH, 2400], BF16, kind="ExternalInput").ap()
    onat1_d = nc.dram_tensor('onat1', [NCH, 2400], BF16, kind="ExternalInput").ap()
    attr_d = [nc.dram_tensor(f'attr{r}', [NCH, 4 * N], BF16,
                             kind="ExternalInput").ap() for r in range(R)]
    wtail_d = nc.dram_tensor('wtail', [128, CT], BF16, kind="ExternalInput").ap()
    biasF_d = nc.dram_tensor('biasF', [128, 6 + NCH], F32,
                             kind="ExternalInput").ap()
    out_d = nc.dram_tensor('out', [BL, P + 1], F32, kind="ExternalOutput").ap()

    mm = nc.tensor.matmul

    with tile.TileContext(nc) as tc, ExitStack() as ctx:
        cp = ctx.enter_context(tc.tile_pool(name="const", bufs=1))
        pw = ctx.enter_context(tc.tile_pool(name="work", bufs=2))
        pp = ctx.enter_context(tc.tile_pool(name="pp", bufs=2, space="PSUM"))

        zw = cp.tile([128, 512], BF16, name='zw', tag='zw')
        nc.gpsimd.memset(zw[:], 0)

        pA = cp.tile([128, CA], BF16, name='pA', tag='pA')
        p22 = cp.tile([22, C22], BF16, name='p22', tag='p22')
        pS = cp.tile([106, CPS], BF16, name='pS', tag='pS')
        on0 = cp.tile([NCH, 2400], BF16, name='on0', tag='on0')
        on1 = cp.tile([NCH, 2400], BF16, name='on1', tag='on1')
        attr = [cp.tile([NCH, 4 * N], BF16, name=f'attr{r}', tag=f'attr{r}')
                for r in range(R)]
        wT = cp.tile([128, CT], BF16, name='wT', tag='wT')
        bF = cp.tile([128, 6 + NCH], F32, name='bF', tag='bF')

        # DMA plan: spread across all five engine queues
        nc.sync.dma_start(out=p22[:], in_=pack22_d[:])
        nc.sync.dma_start(out=pA[:], in_=packA_d[:])
        nc.sync.dma_start(out=pS[:], in_=pds_d[:])
        nc.sync.dma_start(out=bF[:], in_=biasF_d[:])
        nc.sync.dma_start(out=attr[0][:], in_=attr_d[0][:])
        nc.sync.dma_start(out=attr[2][:], in_=attr_d[2][:])
        nc.sync.dma_start(out=wT[:], in_=wtail_d[:])
        nc.scalar.dma_start(out=attr[1][:], in_=attr_d[1][:])
        nc.gpsimd.dma_start(out=on0[:], in_=onat0_d[:])
        nc.gpsimd.dma_start(out=on1[:], in_=onat1_d[:])
        nc.gpsimd.dma_start(out=attr[3][:], in_=attr_d[3][:])

        xsml = [cp.tile([67, 1024], BF16, name=f'xsml{j}', tag=f'xsml{j}')
                for j in range(4)]
        for j in range(4):
            nc.gpsimd.memset(xsml[j][32:64, :], 0)

        # scalar priming: pull the LEAKY_RELU table load to kernel start
        prim = cp.tile([1, 8], BF16, name='prim', tag='prim')
        nc.scalar.activation(prim[:], zw[0:1, 0:8], AF.Lrelu, alpha=SLOPE)

        # ---- PE warmup on zeros (HAM to K=8/8 while DMAs land) ----
        for w in range(18):
            pwm = pp.tile([128, 512], F32, name=f'pwm{w}', tag='conv')
            mm(pwm[:], zw[:, 0:128], zw[:], start=True, stop=True)

        # ---- l-branch max over t (DVE, early) ----
        lm = []
        for b in range(BL):
            t = cp.tile([NCH, 12], F32, name=f'lm{b}', tag=f'lm{b}')
            src = on0 if b < 4 else on1
            nc.vector.tensor_reduce(
                t[:],
                src[:, (b % 4) * 600:(b % 4 + 1) * 600].rearrange(
                    "p (c k t) -> p c k t", c=4, k=C0),
                axis=AX.X, op=ALU.max)
            lm.append(t)

        # ---- conv pairs (PE + ACT; no l-branch deps here) ----
        for j in range(4):
            psA = pp.tile([128, 1024], F32, name=f'psA{j}', tag='conv')
            psB = pp.tile([106, 1024], F32, name=f'psB{j}', tag='conv')
            for i in range(2):
                b = 2 * j + i
                rh = pA[:, OA_RH + b * N: OA_RH + (b + 1) * N]
                rl = p22[:, O22_RL + b * N: O22_RL + (b + 1) * N]
                co = i * 512
                mm(psA[:, co:co + N], pA[:, OA_W1:OA_W1 + 128], rh,
                   start=True, stop=False)
                mm(psA[:, co:co + N], p22[:, O22_W1:O22_W1 + 128], rl,
                   start=False, stop=True)
                mm(psB[:, co:co + N], pA[:, OA_W1 + 128:OA_W1 + 234], rh,
                   start=True, stop=False)
                mm(psB[:, co:co + N], p22[:, O22_W1 + 128:O22_W1 + 234], rl,
                   start=False, stop=True)
            a1 = pw.tile([128, 1024], BF16, name=f's1a{j}', tag='s1a')
            b1 = pw.tile([106, 1024], BF16, name=f's1b{j}', tag='s1b')
            nc.scalar.activation(a1[:], psA[:], AF.Lrelu,
                                 bias=bF[0:128, 0:1], alpha=SLOPE)
            nc.scalar.activation(b1[:], psB[:], AF.Lrelu,
                                 bias=bF[0:106, 1:2], alpha=SLOPE)

            psC = pp.tile([40, 1024], F32, name=f'psC{j}', tag='conv')
            for i in range(2):
                co = i * 512
                mm(psC[:, co:co + N], pA[:, OA_C2K1:OA_C2K1 + 40],
                   a1[:, co:co + N], start=True, stop=False)
                mm(psC[:, co:co + N], pS[:, OPS_C2K2:OPS_C2K2 + 40],
                   b1[:, co:co + N], start=False, stop=True)
            nc.scalar.activation(xsml[j][0:40, :], psC[:], AF.Lrelu,
                                 bias=bF[0:40, 2:3], alpha=SLOPE)

        # ---- l-branch transposes (after conv so conv never waits onat) ----
        def l_pair(j):
            lt = pp.tile([3, 1024], F32, name=f'lt{j}', tag='conv')
            for i in range(2):
                for c in range(4):
                    nc.tensor.transpose(
                        lt[0:3, i * 512 + c * NCH: i * 512 + (c + 1) * NCH],
                        lm[2 * j + i][:, c * 3:(c + 1) * 3],
                        bF[0:NCH, 6:6 + NCH])
            nc.scalar.activation(xsml[j][64:67, :], lt[:], AF.Lrelu,
                                 alpha=SLOPE)

        hsb = [[None, None] for _ in range(BL)]

        def h_pair(j):
            for i in range(2):
                b = 2 * j + i
                for half in range(2):
                    ph = pp.tile([NCH, 344], F32, name=f'ph{b}_{half}', tag='h')
                    for c2 in range(2):
                        c = half * 2 + c2
                        mm(ph[:, c2 * 172:(c2 + 1) * 172],
                           xsml[j][0:67, i * 512 + c * NCH: i * 512 + (c + 1) * NCH],
                           pA[0:67, OA_WALL:OA_WALL + 172],
                           start=True, stop=True)
                    t = cp.tile([NCH, 344], BF16, name=f'hsb{b}_{half}',
                                tag=f'hsb{b}_{half}')
                    nc.vector.tensor_copy(t[:], ph[:])
                    hsb[b][half] = t

        xg = [None] * 4

        def agg_pair(j):
            pg = pp.tile([107, N], F32, name=f'pg{j}', tag='agg')
            mm(pg[:], zw[0:1, 0:107], zw[0:1, 0:N], start=True, stop=False)
            mm(pg[0:43, :], pA[0:67, OA_WROOT:OA_WROOT + 43],
               xsml[j][0:67, 0:N], start=False, stop=False)
            mm(pg[64:107, :], pA[0:67, OA_WROOT:OA_WROOT + 43],
               xsml[j][0:67, 512:512 + N], start=False, stop=False,
               tile_position=(0, 64))
            for r in (1, 0, 2, 3):
                for c in range(4):
                    att = attr[r][:, c * N:(c + 1) * N]
                    mm(pg[0:43, :],
                       hsb[2 * j][c // 2][:, (c % 2) * 172 + r * 43:
                                          (c % 2) * 172 + (r + 1) * 43],
                       att, start=False, stop=False)
                    mm(pg[64:107, :],
                       hsb[2 * j + 1][c // 2][:, (c % 2) * 172 + r * 43:
                                              (c % 2) * 172 + (r + 1) * 43],
                       att, start=False, stop=(r == 3 and c == 3),
                       tile_position=(0, 64))
            x = cp.tile([107, N], BF16, name=f'xg{j}', tag=f'xg{j}')
            nc.scalar.activation(x[:], pg[:], AF.Lrelu,
                                 bias=bF[0:107, 3:4], alpha=SLOPE)
            xg[j] = x

        zpr = [None] * 4

        def z_pair(j):
            zp = pp.tile([2, N], F32, name=f'zp{j}', tag='agg')
            mm(zp[:], pA[0:107, OA_WZG2:OA_WZG2 + 2], xg[j][:],
               start=True, stop=False)
            mm(zp[0:1, :], pA[0:67, OA_WZPT:OA_WZPT + 1],
               xsml[j][0:67, 0:N], start=False, stop=False)
            mm(zp[:], pA[0:67, OA_WZPT2:OA_WZPT2 + 2],
               xsml[j][0:67, 512:512 + N], start=False, stop=True)
            z = pw.tile([2, N], BF16, name=f'zpr{j}', tag='zpr')
            nc.vector.tensor_copy(z[:], zp[:])
            zpr[j] = z

        ztsb = cp.tile([NCH, 32], BF16, name='ztsb', tag='ztsb')

        def zt_pair(j):
            ztq = pp.tile([NCH, 8], BF16, name=f'ztq{j}', tag='agg')
            for c in range(4):
                nc.tensor.transpose(
                    ztq[:, c * 2:(c + 1) * 2],
                    zpr[j][0:2, c * NCH:(c + 1) * NCH],
                    pS[0:2, OPS_ID8:OPS_ID8 + 2])
            nc.vector.tensor_copy(
                ztsb[:].rearrange("p (c b) -> p c b", c=4)[:, :, 2 * j:2 * j + 2],
                ztq[:].rearrange("p (c b) -> p c b", c=4))

        l_pair(0)
        l_pair(1)
        h_pair(0)
        l_pair(2)
        h_pair(1)
        l_pair(3)
        h_pair(2)
        h_pair(3)
        agg_pair(0)
        agg_pair(1)
        z_pair(0)
        agg_pair(2)
        z_pair(1)
        zt_pair(0)
        agg_pair(3)
        z_pair(2)
        zt_pair(1)
        z_pair(3)
        zt_pair(2)
        zt_pair(3)

        # ---- actor head ----
        pg1 = pp.tile([H, BL], F32, name='pg1', tag='agg')
        for c in range(8):
            rhs = (ztsb[:, c * 8:(c + 1) * 8] if c < 4 else
                   wT[0:NCH, OT_ATS + (c - 4) * 8: OT_ATS + (c - 3) * 8])
            mm(pg1[:], wT[0:NCH, OT_W1C + c * H: OT_W1C + (c + 1) * H], rhs,
               start=(c == 0), stop=(c == 7))
        g1 = cp.tile([H, BL], BF16, name='g1', tag='g1')
        nc.scalar.activation(g1[:], pg1[:], AF.Relu, bias=bF[0:128, 4:5])
        pg2 = pp.tile([H, BL], F32, name='pg2', tag='agg')
        mm(pg2[:], pA[:, OA_AW2:OA_AW2 + 128], g1[:], start=True, stop=True)
        g2 = cp.tile([H, BL], BF16, name='g2', tag='g2')
        nc.scalar.activation(g2[:], pg2[:], AF.Relu, bias=bF[0:128, 5:6])

        po = pp.tile([BL, P + 1], F32, name='po', tag='agg')
        mm(po[:], g2[:], wT[:, OT_AW3:OT_AW3 + 501], start=True, stop=False)
        mm(po[:], pA[0:1, OA_ONES:OA_ONES + 8],
           pA[0:1, OA_B3R:OA_B3R + 501], start=False, stop=True)

        mx = pw.tile([BL, 1], F32, name='mx', tag='mx')
        nc.vector.tensor_reduce(mx[:], po[:], axis=AX.X, op=ALU.max)
        sh = pw.tile([BL, P + 1], F32, name='sh', tag='sh')
        nc.vector.tensor_scalar(sh[:], po[:], mx[:, 0:1], None,
                                op0=ALU.subtract)
        ex = pw.tile([BL, P + 1], F32, name='ex', tag='ex')
        sm = pw.tile([BL, 1], F32, name='sm', tag='sm')
        nc.scalar.activation(ex[:], sh[:], AF.Exp, accum_out=sm[:, 0:1])
        rc = pw.tile([BL, 1], F32, name='rc', tag='rc')
        nc.vector.reciprocal(rc[:], sm[:])
        res = pw.tile([BL, P + 1], F32, name='res', tag='res')
        nc.vector.tensor_scalar(res[:], ex[:], rc[:, 0:1], None, op0=ALU.mult)
        nc.sync.dma_start(out=out_d[:], in_=res[:])

    nc.compile()
    return nc


def _get_nc():
    if 'nc' not in _CACHE:
        _CACHE['nc'] = _build_nc()
    return _CACHE['nc']


# ============================ entry point ============================

def _shard_inputs(inputs):
    folded = _host_fold(inputs)
    obs = np.asarray(inputs['observation'], np.float32)
    action = np.asarray(inputs['action'], np.float32)
    obs_t = np.ascontiguousarray(obs.transpose(0, 1, 3, 2)).reshape(B, 150, N)

    in_maps = []
    for i in range(NCORES):
        bs = slice(i * BL, (i + 1) * BL)
        ot = obs_t[bs]
        rh = ot[:, 0:128, :].transpose(1, 0, 2).reshape(128, BL * N)
        rl = ot[:, 128:150, :].transpose(1, 0, 2).reshape(22, BL * N)
        onat = (obs[bs].reshape(BL, C0, 4, NCH, T)
                .transpose(3, 0, 2, 1, 4).reshape(NCH, 4800))
        ats = (action[bs, 1:].reshape(BL, 4, NCH)
               .transpose(2, 1, 0).reshape(NCH, 32))
        wt = folded['wt_const'].copy()
        wt[0:NCH, OT_ATS:OT_ATS + 32] = ats.astype(BF)
        in_maps.append({
            'packA': np.concatenate([folded['pa_const'], rh.astype(BF)], axis=1),
            'pack22': np.concatenate([folded['p22_const'], rl.astype(BF)], axis=1),
            'pdsmall': folded['ps_const'],
            'onat0': np.ascontiguousarray(onat[:, 0:2400]).astype(BF),
            'onat1': np.ascontiguousarray(onat[:, 2400:4800]).astype(BF),
            'attr0': folded['attr'][0], 'attr1': folded['attr'][1],
            'attr2': folded['attr'][2], 'attr3': folded['attr'][3],
            'wtail': wt, 'biasF': folded['biasf'],
        })
    return in_maps


def kernel(**inputs) -> np.ndarray:
    from concourse.bass_utils import run_bass_kernel_spmd

    in_maps = _shard_inputs(inputs)
    nc = _get_nc()
    res = run_bass_kernel_spmd(nc, in_maps, list(range(NCORES)))
    return np.concatenate([np.asarray(r['out'], np.float32)
                           for r in res.results], axis=0)


# revision 9
# speedup vs baseline: 1.0246x; 1.0079x over previous
"""Trainium2 Bass kernel for nn_CustomGPM (multi-scale temporal CNN + RGCN + actor head).

v3: bf16 datapath, DMA spread over all 5 engine queues with per-relation
adjacency chunks, DMA-independent PE warmup + scalar table priming,
pair-packed conv psums/activations, col-tiled RGCN aggregation, and a
DMA-free z-transpose tail.

Layout per core (BL=8 batch elems, 4 pairs):
  xsml[j] [67, 1024] bf16: rows 0:20 s-conv2, 20:40 m-conv2, 40:64 zero,
    64:67 l(max_t); cols b0 0:500, b1 512:1012 (bank-aligned regions)
  hsb[b][i] [125, 344] bf16: node-on-partition H = x^T W_rel, chunks 2i,2i+1
  agg psum [107, 500]: rows 0:43 b0 feats, 64:107 b1 (col-tiled matmuls)
"""

import numpy as np
import ml_dtypes

BF = ml_dtypes.bfloat16

# ---------------- problem constants (hardcoded per spec) ----------------
B = 64
NCORES = 8
BL = B // NCORES          # 8 per core, 4 pairs
C0, N, T, R, P, H = 3, 500, 50, 4, 500, 128
CF = 20
F = 2 * CF + C0           # 43
NCH = 125
TS1, TM1 = 48, 30
SLOPE = 0.01
EPS = 1e-5

# packA (bf16, 128 partitions) column offsets
OA_W1 = 0                 # [150 -> 128+22 split, 234] merged conv1 band
OA_C2K1 = 234             # [128, 40]
OA_AW2 = OA_C2K1 + 40     # [128, 128]
OA_WALL = OA_AW2 + 128    # [67, 172]
OA_WROOT = OA_WALL + 172  # [67, 43]
OA_WZPT = OA_WROOT + 43   # [67, 1]
OA_WZPT2 = OA_WZPT + 1    # [67, 2]
OA_WZG2 = OA_WZPT2 + 2    # [107, 2]
OA_B3R = OA_WZG2 + 2      # [1, 501]
OA_ONES = OA_B3R + 501    # [1, 8]
OA_RH = OA_ONES + 8       # [128, 4000] obs rows 0:128, cols b*500+n
CA = OA_RH + 4000

# pack22 (bf16, 22 partitions)
O22_W1 = 0                # [22, 234] conv1 band rows 128:150
O22_RL = 234              # [22, 4000] obs rows 128:150
C22 = O22_RL + 4000

# pdsmall (bf16, 106 partitions): conv2 K-tile2 + identity
OPS_C2K2 = 0              # [106, 40]
OPS_ID8 = 40              # [8, 8] identity
CPS = 48

# wtail (bf16, 128 partitions): tail-phase weights
OT_AW3 = 0                # [128, 501]
OT_W1C = 501              # [125, 1024] fc1 chunks
OT_ATS = OT_W1C + 1024    # [125, 32] action^T (c, b)
CT = OT_ATS + 32

_CACHE = {}


# ======================= host-side parameter folding =======================

def _bn_fold(p):
    g, b, m, v = np.asarray(p, np.float64)
    s = g / np.sqrt(v + EPS)
    return s, b - m * s


def _conv_band_lhsT(w, bias, bn, t_out):
    w = np.asarray(w, np.float64)[:, :, 0, :]
    co, ci, k = w.shape
    s, t_ = _bn_fold(bn)
    w_eff = w * s[:, None, None]
    b_eff = s * np.asarray(bias, np.float64) + t_
    band = np.zeros((co, t_out, ci, T), np.float64)
    for t in range(t_out):
        band[:, t, :, t:t + k] = w_eff
    lhsT = band.reshape(co * t_out, ci * T).T.copy()
    return lhsT, np.repeat(b_eff, t_out)


def _conv2_fold(w, b, bn):
    w = np.asarray(w, np.float64)[:, :, 0, :]
    s, t_ = _bn_fold(bn)
    w_eff = (w * s[:, None, None]).reshape(CF, -1)
    b_eff = s * np.asarray(b, np.float64) + t_
    return w_eff.T.copy(), b_eff


def _pad67(a):
    """[43, X] -> [67, X]: rows 0:40 = a[0:40], 64:67 = a[40:43]."""
    out = np.zeros((67,) + a.shape[1:], np.float64)
    out[0:40] = a[0:40]
    out[64:67] = a[40:43]
    return out


def _host_fold(inp):
    ws1, bs1 = _conv_band_lhsT(inp['sc1_w'], inp['sc1_b'], inp['sbn1'], TS1)
    wm1, bm1 = _conv_band_lhsT(inp['mc1_w'], inp['mc1_b'], inp['mbn1'], TM1)
    w1all = np.concatenate([ws1, wm1], axis=1)              # [150, 234]
    bias_a = bs1[0:128]
    bias_b = np.concatenate([bs1[128:144], bm1])            # [106]

    ws2, bs2 = _conv2_fold(inp['sc2_w'], inp['sc2_b'], inp['sbn2'])  # [144,20]
    wm2, bm2 = _conv2_fold(inp['mc2_w'], inp['mc2_b'], inp['mbn2'])  # [90,20]
    c2k1 = np.zeros((128, 40), np.float64)
    c2k1[:, 0:20] = ws2[0:128]
    c2k2 = np.zeros((106, 40), np.float64)
    c2k2[0:16, 0:20] = ws2[128:144]
    c2k2[16:106, 20:40] = wm2
    bias_c = np.concatenate([bs2, bm2])                     # [40]

    sg, tg = _bn_fold(inp['gbn'])
    w_all = np.concatenate(
        [np.asarray(inp['gw_rel'], np.float64)[r] * sg[None, :]
         for r in range(R)], axis=1)                        # [43, 172]
    w_root = np.asarray(inp['gw_root'], np.float64) * sg[None, :]
    gb_eff = np.asarray(inp['g_b'], np.float64) * sg + tg
    wallt = _pad67(w_all)
    wroott = _pad67(w_root)

    src = np.asarray(inp['edge_index'][0]).astype(np.int64)
    dst = np.asarray(inp['edge_index'][1]).astype(np.int64)
    etype = np.asarray(inp['edge_type']).astype(np.int64)
    a_t = np.zeros((R, N, N), np.float64)                   # [r, src, dst]
    for r in range(R):
        sel = etype == r
        cnt = np.zeros((N, N), np.float64)
        np.add.at(cnt, (dst[sel], src[sel]), 1.0)
        deg = cnt.sum(axis=1)
        a_t[r] = (cnt / np.maximum(deg, 1.0)[:, None]).T
    # per relation: [125, (c, n)] with src chunked on partitions
    attr = [np.ascontiguousarray(
        a_t[r].reshape(4, NCH, N).transpose(1, 0, 2).reshape(NCH, 4 * N)
    ).astype(BF) for r in range(R)]

    a_cw = np.asarray(inp['a_cw'], np.float64)
    a_cb = float(np.asarray(inp['a_cb'], np.float64)[0])
    a_w1 = np.asarray(inp['a_w1'], np.float64)
    sel_nodes = np.asarray(inp['nodes_to_select']).astype(np.int64)
    w_z = a_cw[1:1 + 2 * F]
    wzpt = _pad67(w_z[0:F].reshape(F, 1))
    wzpt2 = np.zeros((67, 2), np.float64)
    wzpt2[:, 1:2] = wzpt
    wzg2 = np.zeros((107, 2), np.float64)
    wzg2[0:43, 0] = w_z[F:]
    wzg2[64:107, 1] = w_z[F:]
    col_g = np.zeros(107, np.float64)
    col_g[0:43] = gb_eff
    col_g[64:107] = gb_eff

    w1z = np.zeros((N, H), np.float64)
    np.add.at(w1z, sel_nodes, a_w1[1:])
    w1a = a_cw[0] * a_w1[1:]
    b1_eff = np.asarray(inp['a_b1'], np.float64) + a_cb * a_w1[1:].sum(axis=0)
    w1cat = np.concatenate([w1z, w1a], axis=0)              # [1000, 128]
    w1c = w1cat.reshape(8, NCH, H).transpose(1, 0, 2).reshape(NCH, 8 * H)

    biasf = np.zeros((128, 6 + NCH), np.float32)
    biasf[0:128, 0] = bias_a
    biasf[0:106, 1] = bias_b
    biasf[0:40, 2] = bias_c
    biasf[0:107, 3] = col_g
    biasf[0:128, 4] = b1_eff
    biasf[0:128, 5] = np.asarray(inp['a_b2'], np.float64)
    biasf[0:NCH, 6:6 + NCH] = np.eye(NCH)                   # f32 transpose id

    pa = np.zeros((128, OA_RH), np.float64)
    pa[:, OA_W1:OA_W1 + 234] = w1all[0:128]
    pa[:, OA_C2K1:OA_C2K1 + 40] = c2k1
    pa[:, OA_AW2:OA_AW2 + 128] = np.asarray(inp['a_w2'], np.float64)
    pa[0:67, OA_WALL:OA_WALL + 172] = wallt
    pa[0:67, OA_WROOT:OA_WROOT + 43] = wroott
    pa[0:67, OA_WZPT:OA_WZPT + 1] = wzpt
    pa[0:67, OA_WZPT2:OA_WZPT2 + 2] = wzpt2
    pa[0:107, OA_WZG2:OA_WZG2 + 2] = wzg2
    pa[0:1, OA_B3R:OA_B3R + 501] = np.asarray(inp['a_b3'], np.float64)
    pa[0:1, OA_ONES:OA_ONES + 8] = 1.0

    ps = np.zeros((106, CPS), np.float64)
    ps[0:106, OPS_C2K2:OPS_C2K2 + 40] = c2k2
    ps[0:8, OPS_ID8:OPS_ID8 + 8] = np.eye(8)

    wt = np.zeros((128, CT), np.float64)
    wt[:, OT_AW3:OT_AW3 + 501] = np.asarray(inp['a_w3'], np.float64)
    wt[0:NCH, OT_W1C:OT_W1C + 1024] = w1c

    return {
        'pa_const': pa.astype(BF), 'p22_const': w1all[128:150].astype(BF),
        'ps_const': ps.astype(BF), 'wt_const': wt.astype(BF),
        'attr': attr, 'biasf': biasf,
    }


# ============================ device kernel ============================

def _build_nc():
    import concourse.bacc as bacc
    import concourse.tile as tile
    import concourse.mybir as mybir
    from contextlib import ExitStack

    F32 = mybir.dt.float32
    BF16 = mybir.dt.bfloat16
    AF = mybir.ActivationFunctionType
    ALU = mybir.AluOpType
    AX = mybir.AxisListType

    nc = bacc.Bacc("TRN2", target_bir_lowering=False, debug=False)

    packA_d = nc.dram_tensor('packA', [128, CA], BF16, kind="ExternalInput").ap()
    pack22_d = nc.dram_tensor('pack22', [22, C22], BF16, kind="ExternalInput").ap()
    pds_d = nc.dram_tensor('pdsmall', [106, CPS], BF16, kind="ExternalInput").ap()
    onat0_d = nc.dram_tensor('onat0', [NCH, 2400], BF16, kind="ExternalInput").ap()
    onat1_d = nc.dram_tensor('onat1', [NCH, 2400], BF16, kind="ExternalInput").ap()
    attr_d = [nc.dram_tensor(f'attr{r}', [NCH, 4 * N], BF16,
                             kind="ExternalInput").ap() for r in range(R)]
    wtail_d = nc.dram_tensor('wtail', [128, CT], BF16, kind="ExternalInput").ap()
    biasF_d = nc.dram_tensor('biasF', [128, 6 + NCH], F32,
                             kind="ExternalInput").ap()
    out_d = nc.dram_tensor('out', [BL, P + 1], F32, kind="ExternalOutput").ap()

    mm = nc.tensor.matmul

    with tile.TileContext(nc) as tc, ExitStack() as ctx:
        cp = ctx.enter_context(tc.tile_pool(name="const", bufs=1))
        pw = ctx.enter_context(tc.tile_pool(name="work", bufs=2))
        pp = ctx.enter_context(tc.tile_pool(name="pp", bufs=2, space="PSUM"))

        zw = cp.tile([128, 512], BF16, name='zw', tag='zw')
        nc.gpsimd.memset(zw[:], 0)

        pA = cp.tile([128, CA], BF16, name='pA', tag='pA')
        p22 = cp.tile([22, C22], BF16, name='p22', tag='p22')
        pS = cp.tile([106, CPS], BF16, name='pS', tag='pS')
        on0 = cp.tile([NCH, 2400], BF16, name='on0', tag='on0')
        on1 = cp.tile([NCH, 2400], BF16, name='on1', tag='on1')
        attr = [cp.tile([NCH, 4 * N], BF16, name=f'attr{r}', tag=f'attr{r}')
                for r in range(R)]
        wT = cp.tile([128, CT], BF16, name='wT', tag='wT')
        bF = cp.tile([128, 6 + NCH], F32, name='bF', tag='bF')

        # DMA plan: spread across all five engine queues
        # phase 1: conv-critical packs get the full HBM bandwidth
        nc.sync.dma_start(out=p22[:], in_=pack22_d[:])
        nc.sync.dma_start(out=pA[:], in_=packA_d[:])
        nc.sync.dma_start(out=pS[:], in_=pds_d[:])
        nc.sync.dma_start(out=bF[:], in_=biasF_d[:])
        # phase 2+: stage behind packA's arrival via junk reads of pA so the
        # early transfers are not starved by concurrent queue traffic
        jnkS = cp.tile([1, 64], BF16, name='jnkS', tag='jnkS')
        nc.sync.dma_start(out=jnkS[:], in_=pA[0:1, 0:64])
        nc.sync.dma_start(out=attr[0][:], in_=attr_d[0][:])
        nc.sync.dma_start(out=attr[2][:], in_=attr_d[2][:])
        nc.sync.dma_start(out=wT[:], in_=wtail_d[:])

        xsml = [cp.tile([67, 1024], BF16, name=f'xsml{j}', tag=f'xsml{j}')
                for j in range(4)]
        for j in range(4):
            nc.gpsimd.memset(xsml[j][32:64, :], 0)

        # scalar priming: pull the LEAKY_RELU table load to kernel start
        prim = cp.tile([1, 8], BF16, name='prim', tag='prim')
        nc.scalar.activation(prim[:], zw[0:1, 0:8], AF.Lrelu, alpha=SLOPE)
        jnkA = cp.tile([1, 8], BF16, name='jnkA', tag='jnkA')
        nc.scalar.copy(jnkA[:], pA[0:1, 0:8])
        nc.scalar.dma_start(out=attr[1][:], in_=attr_d[1][:])
        jnkG = cp.tile([1, 64], BF16, name='jnkG', tag='jnkG')
        nc.gpsimd.tensor_copy(jnkG[:], pA[0:1, 0:64])
        nc.gpsimd.dma_start(out=on0[:], in_=onat0_d[:])
        nc.gpsimd.dma_start(out=on1[:], in_=onat1_d[:])
        nc.gpsimd.dma_start(out=attr[3][:], in_=attr_d[3][:])

        # ---- PE warmup on zeros (HAM to K=8/8 while DMAs land) ----
        for w in range(18):
            pwm = pp.tile([128, 512], F32, name=f'pwm{w}', tag='conv')
            mm(pwm[:], zw[:, 0:128], zw[:], start=True, stop=True)

        # ---- l-branch max over t (DVE, early) ----
        lm = []
        for b in range(BL):
            t = cp.tile([NCH, 12], F32, name=f'lm{b}', tag=f'lm{b}')
            src = on0 if b < 4 else on1
            nc.vector.tensor_reduce(
                t[:],
                src[:, (b % 4) * 600:(b % 4 + 1) * 600].rearrange(
                    "p (c k t) -> p c k t", c=4, k=C0),
                axis=AX.X, op=ALU.max)
            lm.append(t)

        # ---- conv pairs (PE + ACT; no l-branch deps here) ----
        for j in range(4):
            psA = pp.tile([128, 1024], F32, name=f'psA{j}', tag='conv')
            psB = pp.tile([106, 1024], F32, name=f'psB{j}', tag='conv')
            for i in range(2):
                b = 2 * j + i
                rh = pA[:, OA_RH + b * N: OA_RH + (b + 1) * N]
                rl = p22[:, O22_RL + b * N: O22_RL + (b + 1) * N]
                co = i * 512
                mm(psA[:, co:co + N], pA[:, OA_W1:OA_W1 + 128], rh,
                   start=True, stop=False)
                mm(psA[:, co:co + N], p22[:, O22_W1:O22_W1 + 128], rl,
                   start=False, stop=True)
                mm(psB[:, co:co + N], pA[:, OA_W1 + 128:OA_W1 + 234], rh,
                   start=True, stop=False)
                mm(psB[:, co:co + N], p22[:, O22_W1 + 128:O22_W1 + 234], rl,
                   start=False, stop=True)
            a1 = pw.tile([128, 1024], BF16, name=f's1a{j}', tag='s1a')
            b1 = pw.tile([106, 1024], BF16, name=f's1b{j}', tag='s1b')
            nc.scalar.activation(a1[:], psA[:], AF.Lrelu,
                                 bias=bF[0:128, 0:1], alpha=SLOPE)
            nc.scalar.activation(b1[:], psB[:], AF.Lrelu,
                                 bias=bF[0:106, 1:2], alpha=SLOPE)

            psC = pp.tile([40, 1024], F32, name=f'psC{j}', tag='conv')
            for i in range(2):
                co = i * 512
                mm(psC[:, co:co + N], pA[:, OA_C2K1:OA_C2K1 + 40],
                   a1[:, co:co + N], start=True, stop=False)
                mm(psC[:, co:co + N], pS[:, OPS_C2K2:OPS_C2K2 + 40],
                   b1[:, co:co + N], start=False, stop=True)
            nc.scalar.activation(xsml[j][0:40, :], psC[:], AF.Lrelu,
                                 bias=bF[0:40, 2:3], alpha=SLOPE)

        # ---- l-branch transposes (after conv so conv never waits onat) ----
        def l_pair(j):
            lt = pp.tile([3, 1024], F32, name=f'lt{j}', tag='conv')
            for i in range(2):
                for c in range(4):
                    nc.tensor.transpose(
                        lt[0:3, i * 512 + c * NCH: i * 512 + (c + 1) * NCH],
                        lm[2 * j + i][:, c * 3:(c + 1) * 3],
                        bF[0:NCH, 6:6 + NCH])
            nc.scalar.activation(xsml[j][64:67, :], lt[:], AF.Lrelu,
                                 alpha=SLOPE)

        hsb = [[None, None] for _ in range(BL)]

        def h_pair(j):
            for i in range(2):
                b = 2 * j + i
                for half in range(2):
                    ph = pp.tile([NCH, 344], F32, name=f'ph{b}_{half}', tag='h')
                    for c2 in range(2):
                        c = half * 2 + c2
                        mm(ph[:, c2 * 172:(c2 + 1) * 172],
                           xsml[j][0:67, i * 512 + c * NCH: i * 512 + (c + 1) * NCH],
                           pA[0:67, OA_WALL:OA_WALL + 172],
                           start=True, stop=True)
                    t = cp.tile([NCH, 344], BF16, name=f'hsb{b}_{half}',
                                tag=f'hsb{b}_{half}')
                    nc.vector.tensor_copy(t[:], ph[:])
                    hsb[b][half] = t

        xg = [None] * 4

        def agg_pair(j):
            pg = pp.tile([107, N], F32, name=f'pg{j}', tag='agg')
            mm(pg[:], zw[0:1, 0:107], zw[0:1, 0:N], start=True, stop=False)
            mm(pg[0:43, :], pA[0:67, OA_WROOT:OA_WROOT + 43],
               xsml[j][0:67, 0:N], start=False, stop=False)
            mm(pg[64:107, :], pA[0:67, OA_WROOT:OA_WROOT + 43],
               xsml[j][0:67, 512:512 + N], start=False, stop=False,
               tile_position=(0, 64))
            for r in (1, 0, 2, 3):
                for c in range(4):
                    att = attr[r][:, c * N:(c + 1) * N]
                    mm(pg[0:43, :],
                       hsb[2 * j][c // 2][:, (c % 2) * 172 + r * 43:
                                          (c % 2) * 172 + (r + 1) * 43],
                       att, start=False, stop=False)
                    mm(pg[64:107, :],
                       hsb[2 * j + 1][c // 2][:, (c % 2) * 172 + r * 43:
                                              (c % 2) * 172 + (r + 1) * 43],
                       att, start=False, stop=(r == 3 and c == 3),
                       tile_position=(0, 64))
            x = cp.tile([107, N], BF16, name=f'xg{j}', tag=f'xg{j}')
            nc.scalar.activation(x[:], pg[:], AF.Lrelu,
                                 bias=bF[0:107, 3:4], alpha=SLOPE)
            xg[j] = x

        zpr = [None] * 4

        def z_pair(j):
            zp = pp.tile([2, N], F32, name=f'zp{j}', tag='agg')
            mm(zp[:], pA[0:107, OA_WZG2:OA_WZG2 + 2], xg[j][:],
               start=True, stop=False)
            mm(zp[0:1, :], pA[0:67, OA_WZPT:OA_WZPT + 1],
               xsml[j][0:67, 0:N], start=False, stop=False)
            mm(zp[:], pA[0:67, OA_WZPT2:OA_WZPT2 + 2],
               xsml[j][0:67, 512:512 + N], start=False, stop=True)
            z = pw.tile([2, N], BF16, name=f'zpr{j}', tag='zpr')
            nc.vector.tensor_copy(z[:], zp[:])
            zpr[j] = z

        ztsb = cp.tile([NCH, 32], BF16, name='ztsb', tag='ztsb')

        def zt_pair(j):
            ztq = pp.tile([NCH, 8], BF16, name=f'ztq{j}', tag='agg')
            for c in range(4):
                nc.tensor.transpose(
                    ztq[:, c * 2:(c + 1) * 2],
                    zpr[j][0:2, c * NCH:(c + 1) * NCH],
                    pS[0:2, OPS_ID8:OPS_ID8 + 2])
            nc.vector.tensor_copy(
                ztsb[:].rearrange("p (c b) -> p c b", c=4)[:, :, 2 * j:2 * j + 2],
                ztq[:].rearrange("p (c b) -> p c b", c=4))

        l_pair(0)
        l_pair(1)
        h_pair(0)
        l_pair(2)
        h_pair(1)
        l_pair(3)
        h_pair(2)
        h_pair(3)
        agg_pair(0)
        agg_pair(1)
        z_pair(0)
        agg_pair(2)
        z_pair(1)
        zt_pair(0)
        agg_pair(3)
        z_pair(2)
        zt_pair(1)
        z_pair(3)
        zt_pair(2)
        zt_pair(3)

        # ---- actor head ----
        pg1 = pp.tile([H, BL], F32, name='pg1', tag='agg')
        for c in range(8):
            rhs = (ztsb[:, c * 8:(c + 1) * 8] if c < 4 else
                   wT[0:NCH, OT_ATS + (c - 4) * 8: OT_ATS + (c - 3) * 8])
            mm(pg1[:], wT[0:NCH, OT_W1C + c * H: OT_W1C + (c + 1) * H], rhs,
               start=(c == 0), stop=(c == 7))
        g1 = cp.tile([H, BL], BF16, name='g1', tag='g1')
        nc.scalar.activation(g1[:], pg1[:], AF.Relu, bias=bF[0:128, 4:5])
        pg2 = pp.tile([H, BL], F32, name='pg2', tag='agg')
        mm(pg2[:], pA[:, OA_AW2:OA_AW2 + 128], g1[:], start=True, stop=True)
        g2 = cp.tile([H, BL], BF16, name='g2', tag='g2')
        nc.scalar.activation(g2[:], pg2[:], AF.Relu, bias=bF[0:128, 5:6])

        po = pp.tile([BL, P + 1], F32, name='po', tag='agg')
        mm(po[:], g2[:], wT[:, OT_AW3:OT_AW3 + 501], start=True, stop=False)
        mm(po[:], pA[0:1, OA_ONES:OA_ONES + 8],
           pA[0:1, OA_B3R:OA_B3R + 501], start=False, stop=True)

        mx = pw.tile([BL, 1], F32, name='mx', tag='mx')
        nc.vector.tensor_reduce(mx[:], po[:], axis=AX.X, op=ALU.max)
        sh = pw.tile([BL, P + 1], F32, name='sh', tag='sh')
        nc.vector.tensor_scalar(sh[:], po[:], mx[:, 0:1], None,
                                op0=ALU.subtract)
        ex = pw.tile([BL, P + 1], F32, name='ex', tag='ex')
        sm = pw.tile([BL, 1], F32, name='sm', tag='sm')
        nc.scalar.activation(ex[:], sh[:], AF.Exp, accum_out=sm[:, 0:1])
        rc = pw.tile([BL, 1], F32, name='rc', tag='rc')
        nc.vector.reciprocal(rc[:], sm[:])
        res = pw.tile([BL, P + 1], F32, name='res', tag='res')
        nc.vector.tensor_scalar(res[:], ex[:], rc[:, 0:1], None, op0=ALU.mult)
        nc.sync.dma_start(out=out_d[:], in_=res[:])

    nc.compile()
    return nc


def _get_nc():
    if 'nc' not in _CACHE:
        _CACHE['nc'] = _build_nc()
    return _CACHE['nc']


# ============================ entry point ============================

def _shard_inputs(inputs):
    folded = _host_fold(inputs)
    obs = np.asarray(inputs['observation'], np.float32)
    action = np.asarray(inputs['action'], np.float32)
    obs_t = np.ascontiguousarray(obs.transpose(0, 1, 3, 2)).reshape(B, 150, N)

    in_maps = []
    for i in range(NCORES):
        bs = slice(i * BL, (i + 1) * BL)
        ot = obs_t[bs]
        rh = ot[:, 0:128, :].transpose(1, 0, 2).reshape(128, BL * N)
        rl = ot[:, 128:150, :].transpose(1, 0, 2).reshape(22, BL * N)
        onat = (obs[bs].reshape(BL, C0, 4, NCH, T)
                .transpose(3, 0, 2, 1, 4).reshape(NCH, 4800))
        ats = (action[bs, 1:].reshape(BL, 4, NCH)
               .transpose(2, 1, 0).reshape(NCH, 32))
        wt = folded['wt_const'].copy()
        wt[0:NCH, OT_ATS:OT_ATS + 32] = ats.astype(BF)
        in_maps.append({
            'packA': np.concatenate([folded['pa_const'], rh.astype(BF)], axis=1),
            'pack22': np.concatenate([folded['p22_const'], rl.astype(BF)], axis=1),
            'pdsmall': folded['ps_const'],
            'onat0': np.ascontiguousarray(onat[:, 0:2400]).astype(BF),
            'onat1': np.ascontiguousarray(onat[:, 2400:4800]).astype(BF),
            'attr0': folded['attr'][0], 'attr1': folded['attr'][1],
            'attr2': folded['attr'][2], 'attr3': folded['attr'][3],
            'wtail': wt, 'biasF': folded['biasf'],
        })
    return in_maps


def kernel(**inputs) -> np.ndarray:
    from concourse.bass_utils import run_bass_kernel_spmd

    in_maps = _shard_inputs(inputs)
    nc = _get_nc()
    res = run_bass_kernel_spmd(nc, in_maps, list(range(NCORES)))
    return np.concatenate([np.asarray(r['out'], np.float32)
                           for r in res.results], axis=0)


# revision 10
# speedup vs baseline: 1.1517x; 1.1240x over previous
"""Trainium2 Bass kernel for nn_CustomGPM (multi-scale temporal CNN + RGCN + actor head).

v3: bf16 datapath, DMA spread over all 5 engine queues with per-relation
adjacency chunks, DMA-independent PE warmup + scalar table priming,
pair-packed conv psums/activations, col-tiled RGCN aggregation, and a
DMA-free z-transpose tail.

Layout per core (BL=8 batch elems, 4 pairs):
  xsml[j] [67, 1024] bf16: rows 0:20 s-conv2, 20:40 m-conv2, 40:64 zero,
    64:67 l(max_t); cols b0 0:500, b1 512:1012 (bank-aligned regions)
  hsb[b][i] [125, 344] bf16: node-on-partition H = x^T W_rel, chunks 2i,2i+1
  agg psum [107, 500]: rows 0:43 b0 feats, 64:107 b1 (col-tiled matmuls)
"""

import numpy as np
import ml_dtypes

BF = ml_dtypes.bfloat16

# ---------------- problem constants (hardcoded per spec) ----------------
B = 64
NCORES = 8
BL = B // NCORES          # 8 per core, 4 pairs
C0, N, T, R, P, H = 3, 500, 50, 4, 500, 128
CF = 20
F = 2 * CF + C0           # 43
NCH = 125
TS1, TM1 = 48, 30
SLOPE = 0.01
EPS = 1e-5

# packA (bf16, 128 partitions) column offsets
OA_W1 = 0                 # [150 -> 128+22 split, 234] merged conv1 band
OA_C2K1 = 234             # [128, 40]
OA_AW2 = OA_C2K1 + 40     # [128, 128]
OA_WALL = OA_AW2 + 128    # [67, 172]
OA_WROOT = OA_WALL + 172  # [67, 43]
OA_WZPT = OA_WROOT + 43   # [67, 1]
OA_WZPT2 = OA_WZPT + 1    # [67, 2]
OA_WZG2 = OA_WZPT2 + 2    # [107, 2]
OA_B3R = OA_WZG2 + 2      # [1, 501]
OA_ONES = OA_B3R + 501    # [1, 8]
OA_RH = OA_ONES + 8       # [128, 4000] obs rows 0:128, cols b*500+n
CA = OA_RH + 4000

# pack22 (bf16, 22 partitions)
O22_W1 = 0                # [22, 234] conv1 band rows 128:150
O22_RL = 234              # [22, 4000] obs rows 128:150
C22 = O22_RL + 4000

# pdsmall (bf16, 106 partitions): conv2 K-tile2 + identity
OPS_C2K2 = 0              # [106, 40]
OPS_ID8 = 40              # [8, 8] identity
CPS = 48

# wtail (bf16, 128 partitions): tail-phase weights
OT_AW3 = 0                # [128, 501]
OT_W1C = 501              # [125, 1024] fc1 chunks
OT_ATS = OT_W1C + 1024    # [125, 32] action^T (c, b)
CT = OT_ATS + 32

_CACHE = {}


# ======================= host-side parameter folding =======================

def _bn_fold(p):
    g, b, m, v = np.asarray(p, np.float64)
    s = g / np.sqrt(v + EPS)
    return s, b - m * s


def _conv_band_lhsT(w, bias, bn, t_out):
    w = np.asarray(w, np.float64)[:, :, 0, :]
    co, ci, k = w.shape
    s, t_ = _bn_fold(bn)
    w_eff = w * s[:, None, None]
    b_eff = s * np.asarray(bias, np.float64) + t_
    band = np.zeros((co, t_out, ci, T), np.float64)
    for t in range(t_out):
        band[:, t, :, t:t + k] = w_eff
    lhsT = band.reshape(co * t_out, ci * T).T.copy()
    return lhsT, np.repeat(b_eff, t_out)


def _conv2_fold(w, b, bn):
    w = np.asarray(w, np.float64)[:, :, 0, :]
    s, t_ = _bn_fold(bn)
    w_eff = (w * s[:, None, None]).reshape(CF, -1)
    b_eff = s * np.asarray(b, np.float64) + t_
    return w_eff.T.copy(), b_eff


def _pad67(a):
    """[43, X] -> [67, X]: rows 0:40 = a[0:40], 64:67 = a[40:43]."""
    out = np.zeros((67,) + a.shape[1:], np.float64)
    out[0:40] = a[0:40]
    out[64:67] = a[40:43]
    return out


def _host_fold(inp):
    ws1, bs1 = _conv_band_lhsT(inp['sc1_w'], inp['sc1_b'], inp['sbn1'], TS1)
    wm1, bm1 = _conv_band_lhsT(inp['mc1_w'], inp['mc1_b'], inp['mbn1'], TM1)
    w1all = np.concatenate([ws1, wm1], axis=1)              # [150, 234]
    bias_a = bs1[0:128]
    bias_b = np.concatenate([bs1[128:144], bm1])            # [106]

    ws2, bs2 = _conv2_fold(inp['sc2_w'], inp['sc2_b'], inp['sbn2'])  # [144,20]
    wm2, bm2 = _conv2_fold(inp['mc2_w'], inp['mc2_b'], inp['mbn2'])  # [90,20]
    c2k1 = np.zeros((128, 40), np.float64)
    c2k1[:, 0:20] = ws2[0:128]
    c2k2 = np.zeros((106, 40), np.float64)
    c2k2[0:16, 0:20] = ws2[128:144]
    c2k2[16:106, 20:40] = wm2
    bias_c = np.concatenate([bs2, bm2])                     # [40]

    sg, tg = _bn_fold(inp['gbn'])
    w_all = np.concatenate(
        [np.asarray(inp['gw_rel'], np.float64)[r] * sg[None, :]
         for r in range(R)], axis=1)                        # [43, 172]
    w_root = np.asarray(inp['gw_root'], np.float64) * sg[None, :]
    gb_eff = np.asarray(inp['g_b'], np.float64) * sg + tg
    wallt = _pad67(w_all)
    wroott = _pad67(w_root)

    src = np.asarray(inp['edge_index'][0]).astype(np.int64)
    dst = np.asarray(inp['edge_index'][1]).astype(np.int64)
    etype = np.asarray(inp['edge_type']).astype(np.int64)
    a_t = np.zeros((R, N, N), np.float64)                   # [r, src, dst]
    for r in range(R):
        sel = etype == r
        cnt = np.zeros((N, N), np.float64)
        np.add.at(cnt, (dst[sel], src[sel]), 1.0)
        deg = cnt.sum(axis=1)
        a_t[r] = (cnt / np.maximum(deg, 1.0)[:, None]).T
    # per relation: [125, (c, n)] with src chunked on partitions
    attr = [np.ascontiguousarray(
        a_t[r].reshape(4, NCH, N).transpose(1, 0, 2).reshape(NCH, 4 * N)
    ).astype(BF) for r in range(R)]

    a_cw = np.asarray(inp['a_cw'], np.float64)
    a_cb = float(np.asarray(inp['a_cb'], np.float64)[0])
    a_w1 = np.asarray(inp['a_w1'], np.float64)
    sel_nodes = np.asarray(inp['nodes_to_select']).astype(np.int64)
    w_z = a_cw[1:1 + 2 * F]
    wzpt = _pad67(w_z[0:F].reshape(F, 1))
    wzpt2 = np.zeros((67, 2), np.float64)
    wzpt2[:, 1:2] = wzpt
    wzg2 = np.zeros((107, 2), np.float64)
    wzg2[0:43, 0] = w_z[F:]
    wzg2[64:107, 1] = w_z[F:]
    col_g = np.zeros(107, np.float64)
    col_g[0:43] = gb_eff
    col_g[64:107] = gb_eff

    w1z = np.zeros((N, H), np.float64)
    np.add.at(w1z, sel_nodes, a_w1[1:])
    w1a = a_cw[0] * a_w1[1:]
    b1_eff = np.asarray(inp['a_b1'], np.float64) + a_cb * a_w1[1:].sum(axis=0)
    w1cat = np.concatenate([w1z, w1a], axis=0)              # [1000, 128]
    w1c = w1cat.reshape(8, NCH, H).transpose(1, 0, 2).reshape(NCH, 8 * H)

    biasf = np.zeros((128, 6 + NCH), np.float32)
    biasf[0:128, 0] = bias_a
    biasf[0:106, 1] = bias_b
    biasf[0:40, 2] = bias_c
    biasf[0:107, 3] = col_g
    biasf[0:128, 4] = b1_eff
    biasf[0:128, 5] = np.asarray(inp['a_b2'], np.float64)
    biasf[0:NCH, 6:6 + NCH] = np.eye(NCH)                   # f32 transpose id

    pa = np.zeros((128, OA_RH), np.float64)
    pa[:, OA_W1:OA_W1 + 234] = w1all[0:128]
    pa[:, OA_C2K1:OA_C2K1 + 40] = c2k1
    pa[:, OA_AW2:OA_AW2 + 128] = np.asarray(inp['a_w2'], np.float64)
    pa[0:67, OA_WALL:OA_WALL + 172] = wallt
    pa[0:67, OA_WROOT:OA_WROOT + 43] = wroott
    pa[0:67, OA_WZPT:OA_WZPT + 1] = wzpt
    pa[0:67, OA_WZPT2:OA_WZPT2 + 2] = wzpt2
    pa[0:107, OA_WZG2:OA_WZG2 + 2] = wzg2
    pa[0:1, OA_B3R:OA_B3R + 501] = np.asarray(inp['a_b3'], np.float64)
    pa[0:1, OA_ONES:OA_ONES + 8] = 1.0

    ps = np.zeros((106, CPS), np.float64)
    ps[0:106, OPS_C2K2:OPS_C2K2 + 40] = c2k2
    ps[0:8, OPS_ID8:OPS_ID8 + 8] = np.eye(8)

    wt = np.zeros((128, CT), np.float64)
    wt[:, OT_AW3:OT_AW3 + 501] = np.asarray(inp['a_w3'], np.float64)
    wt[0:NCH, OT_W1C:OT_W1C + 1024] = w1c

    return {
        'pa_const': pa.astype(BF), 'p22_const': w1all[128:150].astype(BF),
        'ps_const': ps.astype(BF), 'wt_const': wt.astype(BF),
        'attr': attr, 'biasf': biasf,
    }


# ============================ device kernel ============================

def _build_nc():
    import concourse.bacc as bacc
    import concourse.tile as tile
    import concourse.mybir as mybir
    from contextlib import ExitStack

    F32 = mybir.dt.float32
    BF16 = mybir.dt.bfloat16
    AF = mybir.ActivationFunctionType
    ALU = mybir.AluOpType
    AX = mybir.AxisListType

    nc = bacc.Bacc("TRN2", target_bir_lowering=False, debug=False)

    packA_d = nc.dram_tensor('packA', [128, CA], BF16, kind="ExternalInput").ap()
    pack22_d = nc.dram_tensor('pack22', [22, C22], BF16, kind="ExternalInput").ap()
    pds_d = nc.dram_tensor('pdsmall', [106, CPS], BF16, kind="ExternalInput").ap()
    onat0_d = nc.dram_tensor('onat0', [NCH, 2400], BF16, kind="ExternalInput").ap()
    onat1_d = nc.dram_tensor('onat1', [NCH, 2400], BF16, kind="ExternalInput").ap()
    attr_d = [nc.dram_tensor(f'attr{r}', [NCH, 4 * N], BF16,
                             kind="ExternalInput").ap() for r in range(R)]
    wtail_d = nc.dram_tensor('wtail', [128, CT], BF16, kind="ExternalInput").ap()
    biasF_d = nc.dram_tensor('biasF', [128, 6 + NCH], F32,
                             kind="ExternalInput").ap()
    out_d = nc.dram_tensor('out', [BL, P + 1], F32, kind="ExternalOutput").ap()

    mm = nc.tensor.matmul

    with tile.TileContext(nc) as tc, ExitStack() as ctx:
        cp = ctx.enter_context(tc.tile_pool(name="const", bufs=1))
        pw = ctx.enter_context(tc.tile_pool(name="work", bufs=2))
        pp = ctx.enter_context(tc.tile_pool(name="pp", bufs=2, space="PSUM"))

        zw = cp.tile([128, 512], BF16, name='zw', tag='zw')
        nc.gpsimd.memset(zw[:], 0)

        pA = cp.tile([128, CA], BF16, name='pA', tag='pA')
        p22 = cp.tile([22, C22], BF16, name='p22', tag='p22')
        pS = cp.tile([106, CPS], BF16, name='pS', tag='pS')
        on0 = cp.tile([NCH, 2400], BF16, name='on0', tag='on0')
        on1 = cp.tile([NCH, 2400], BF16, name='on1', tag='on1')
        attr = [cp.tile([NCH, 4 * N], BF16, name=f'attr{r}', tag=f'attr{r}')
                for r in range(R)]
        wT = cp.tile([128, CT], BF16, name='wT', tag='wT')
        bF = cp.tile([128, 6 + NCH], F32, name='bF', tag='bF')

        # DMA plan: spread across all five engine queues
        # phase 1: conv-critical packs get the full HBM bandwidth
        nc.sync.dma_start(out=p22[:], in_=pack22_d[:])
        nc.sync.dma_start(out=pA[:], in_=packA_d[:])
        nc.sync.dma_start(out=pS[:], in_=pds_d[:])
        nc.sync.dma_start(out=bF[:], in_=biasF_d[:])
        # phase 2+: stage behind packA's arrival via junk reads of pA so the
        # early transfers are not starved by concurrent queue traffic
        nc.sync.dma_start(out=attr[0][0:1, 0:64], in_=pA[0:1, 0:64])
        nc.sync.dma_start(out=attr[0][:], in_=attr_d[0][:])
        nc.sync.dma_start(out=attr[2][0:1, 0:64], in_=pA[0:1, 0:64])
        nc.sync.dma_start(out=attr[2][:], in_=attr_d[2][:])
        nc.sync.dma_start(out=wT[0:1, 0:64], in_=pA[0:1, 0:64])
        nc.sync.dma_start(out=wT[:], in_=wtail_d[:])

        xsml = [cp.tile([67, 1024], BF16, name=f'xsml{j}', tag=f'xsml{j}')
                for j in range(4)]
        for j in range(4):
            nc.gpsimd.memset(xsml[j][32:64, :], 0)

        # scalar priming: pull the LEAKY_RELU table load to kernel start
        prim = cp.tile([1, 8], BF16, name='prim', tag='prim')
        nc.scalar.activation(prim[:], zw[0:1, 0:8], AF.Lrelu, alpha=SLOPE)
        nc.scalar.copy(attr[1][0:1, 0:8].bitcast(BF16), pA[0:1, 0:8])
        nc.scalar.dma_start(out=attr[1][:], in_=attr_d[1][:])
        nc.gpsimd.tensor_copy(on0[0:1, 0:64], pA[0:1, 0:64])
        nc.gpsimd.dma_start(out=on0[:], in_=onat0_d[:])
        nc.gpsimd.tensor_copy(on1[0:1, 0:64], pA[0:1, 0:64])
        nc.gpsimd.dma_start(out=on1[:], in_=onat1_d[:])
        nc.gpsimd.tensor_copy(attr[3][0:1, 0:64], pA[0:1, 0:64])
        nc.gpsimd.dma_start(out=attr[3][:], in_=attr_d[3][:])

        # ---- PE warmup on zeros (HAM to K=8/8 while DMAs land) ----
        for w in range(18):
            pwm = pp.tile([128, 512], F32, name=f'pwm{w}', tag='conv')
            mm(pwm[:], zw[:, 0:128], zw[:], start=True, stop=True)

        # ---- l-branch max over t (DVE, early) ----
        lm = []
        for b in range(BL):
            t = cp.tile([NCH, 12], F32, name=f'lm{b}', tag=f'lm{b}')
            src = on0 if b < 4 else on1
            nc.vector.tensor_reduce(
                t[:],
                src[:, (b % 4) * 600:(b % 4 + 1) * 600].rearrange(
                    "p (c k t) -> p c k t", c=4, k=C0),
                axis=AX.X, op=ALU.max)
            lm.append(t)

        # ---- conv pairs (PE + ACT; no l-branch deps here) ----
        for j in range(4):
            psA = pp.tile([128, 1024], F32, name=f'psA{j}', tag='conv')
            psB = pp.tile([106, 1024], F32, name=f'psB{j}', tag='conv')
            for i in range(2):
                b = 2 * j + i
                rh = pA[:, OA_RH + b * N: OA_RH + (b + 1) * N]
                rl = p22[:, O22_RL + b * N: O22_RL + (b + 1) * N]
                co = i * 512
                mm(psA[:, co:co + N], pA[:, OA_W1:OA_W1 + 128], rh,
                   start=True, stop=False)
                mm(psA[:, co:co + N], p22[:, O22_W1:O22_W1 + 128], rl,
                   start=False, stop=True)
                mm(psB[:, co:co + N], pA[:, OA_W1 + 128:OA_W1 + 234], rh,
                   start=True, stop=False)
                mm(psB[:, co:co + N], p22[:, O22_W1 + 128:O22_W1 + 234], rl,
                   start=False, stop=True)
            a1 = pw.tile([128, 1024], BF16, name=f's1a{j}', tag='s1a')
            b1 = pw.tile([106, 1024], BF16, name=f's1b{j}', tag='s1b')
            nc.scalar.activation(a1[:], psA[:], AF.Lrelu,
                                 bias=bF[0:128, 0:1], alpha=SLOPE)
            nc.scalar.activation(b1[:], psB[:], AF.Lrelu,
                                 bias=bF[0:106, 1:2], alpha=SLOPE)

            psC = pp.tile([40, 1024], F32, name=f'psC{j}', tag='conv')
            for i in range(2):
                co = i * 512
                mm(psC[:, co:co + N], pA[:, OA_C2K1:OA_C2K1 + 40],
                   a1[:, co:co + N], start=True, stop=False)
                mm(psC[:, co:co + N], pS[:, OPS_C2K2:OPS_C2K2 + 40],
                   b1[:, co:co + N], start=False, stop=True)
            nc.scalar.activation(xsml[j][0:40, :], psC[:], AF.Lrelu,
                                 bias=bF[0:40, 2:3], alpha=SLOPE)

        # ---- l-branch transposes (after conv so conv never waits onat) ----
        def l_pair(j):
            lt = pp.tile([3, 1024], F32, name=f'lt{j}', tag='conv')
            for i in range(2):
                for c in range(4):
                    nc.tensor.transpose(
                        lt[0:3, i * 512 + c * NCH: i * 512 + (c + 1) * NCH],
                        lm[2 * j + i][:, c * 3:(c + 1) * 3],
                        bF[0:NCH, 6:6 + NCH])
            nc.scalar.activation(xsml[j][64:67, :], lt[:], AF.Lrelu,
                                 alpha=SLOPE)

        hsb = [[None, None] for _ in range(BL)]

        def h_pair(j):
            for i in range(2):
                b = 2 * j + i
                for half in range(2):
                    ph = pp.tile([NCH, 344], F32, name=f'ph{b}_{half}', tag='h')
                    for c2 in range(2):
                        c = half * 2 + c2
                        mm(ph[:, c2 * 172:(c2 + 1) * 172],
                           xsml[j][0:67, i * 512 + c * NCH: i * 512 + (c + 1) * NCH],
                           pA[0:67, OA_WALL:OA_WALL + 172],
                           start=True, stop=True)
                    t = cp.tile([NCH, 344], BF16, name=f'hsb{b}_{half}',
                                tag=f'hsb{b}_{half}')
                    nc.vector.tensor_copy(t[:], ph[:])
                    hsb[b][half] = t

        xg = [None] * 4

        def agg_pair(j):
            pg = pp.tile([107, N], F32, name=f'pg{j}', tag='agg')
            mm(pg[:], zw[0:1, 0:107], zw[0:1, 0:N], start=True, stop=False)
            mm(pg[0:43, :], pA[0:67, OA_WROOT:OA_WROOT + 43],
               xsml[j][0:67, 0:N], start=False, stop=False)
            mm(pg[64:107, :], pA[0:67, OA_WROOT:OA_WROOT + 43],
               xsml[j][0:67, 512:512 + N], start=False, stop=False,
               tile_position=(0, 64))
            for r in (1, 0, 2, 3):
                for c in range(4):
                    att = attr[r][:, c * N:(c + 1) * N]
                    mm(pg[0:43, :],
                       hsb[2 * j][c // 2][:, (c % 2) * 172 + r * 43:
                                          (c % 2) * 172 + (r + 1) * 43],
                       att, start=False, stop=False)
                    mm(pg[64:107, :],
                       hsb[2 * j + 1][c // 2][:, (c % 2) * 172 + r * 43:
                                              (c % 2) * 172 + (r + 1) * 43],
                       att, start=False, stop=(r == 3 and c == 3),
                       tile_position=(0, 64))
            x = cp.tile([107, N], BF16, name=f'xg{j}', tag=f'xg{j}')
            nc.scalar.activation(x[:], pg[:], AF.Lrelu,
                                 bias=bF[0:107, 3:4], alpha=SLOPE)
            xg[j] = x

        zpr = [None] * 4

        def z_pair(j):
            zp = pp.tile([2, N], F32, name=f'zp{j}', tag='agg')
            mm(zp[:], pA[0:107, OA_WZG2:OA_WZG2 + 2], xg[j][:],
               start=True, stop=False)
            mm(zp[0:1, :], pA[0:67, OA_WZPT:OA_WZPT + 1],
               xsml[j][0:67, 0:N], start=False, stop=False)
            mm(zp[:], pA[0:67, OA_WZPT2:OA_WZPT2 + 2],
               xsml[j][0:67, 512:512 + N], start=False, stop=True)
            z = pw.tile([2, N], BF16, name=f'zpr{j}', tag='zpr')
            nc.vector.tensor_copy(z[:], zp[:])
            zpr[j] = z

        ztsb = cp.tile([NCH, 32], BF16, name='ztsb', tag='ztsb')

        def zt_pair(j):
            ztq = pp.tile([NCH, 8], BF16, name=f'ztq{j}', tag='agg')
            for c in range(4):
                nc.tensor.transpose(
                    ztq[:, c * 2:(c + 1) * 2],
                    zpr[j][0:2, c * NCH:(c + 1) * NCH],
                    pS[0:2, OPS_ID8:OPS_ID8 + 2])
            nc.vector.tensor_copy(
                ztsb[:].rearrange("p (c b) -> p c b", c=4)[:, :, 2 * j:2 * j + 2],
                ztq[:].rearrange("p (c b) -> p c b", c=4))

        l_pair(0)
        l_pair(1)
        h_pair(0)
        l_pair(2)
        h_pair(1)
        l_pair(3)
        h_pair(2)
        h_pair(3)
        agg_pair(0)
        agg_pair(1)
        z_pair(0)
        agg_pair(2)
        z_pair(1)
        zt_pair(0)
        agg_pair(3)
        z_pair(2)
        zt_pair(1)
        z_pair(3)
        zt_pair(2)
        zt_pair(3)

        # ---- actor head ----
        pg1 = pp.tile([H, BL], F32, name='pg1', tag='agg')
        for c in range(8):
            rhs = (ztsb[:, c * 8:(c + 1) * 8] if c < 4 else
                   wT[0:NCH, OT_ATS + (c - 4) * 8: OT_ATS + (c - 3) * 8])
            mm(pg1[:], wT[0:NCH, OT_W1C + c * H: OT_W1C + (c + 1) * H], rhs,
               start=(c == 0), stop=(c == 7))
        g1 = cp.tile([H, BL], BF16, name='g1', tag='g1')
        nc.scalar.activation(g1[:], pg1[:], AF.Relu, bias=bF[0:128, 4:5])
        pg2 = pp.tile([H, BL], F32, name='pg2', tag='agg')
        mm(pg2[:], pA[:, OA_AW2:OA_AW2 + 128], g1[:], start=True, stop=True)
        g2 = cp.tile([H, BL], BF16, name='g2', tag='g2')
        nc.scalar.activation(g2[:], pg2[:], AF.Relu, bias=bF[0:128, 5:6])

        po = pp.tile([BL, P + 1], F32, name='po', tag='agg')
        mm(po[:], g2[:], wT[:, OT_AW3:OT_AW3 + 501], start=True, stop=False)
        mm(po[:], pA[0:1, OA_ONES:OA_ONES + 8],
           pA[0:1, OA_B3R:OA_B3R + 501], start=False, stop=True)

        mx = pw.tile([BL, 1], F32, name='mx', tag='mx')
        nc.vector.tensor_reduce(mx[:], po[:], axis=AX.X, op=ALU.max)
        sh = pw.tile([BL, P + 1], F32, name='sh', tag='sh')
        nc.vector.tensor_scalar(sh[:], po[:], mx[:, 0:1], None,
                                op0=ALU.subtract)
        ex = pw.tile([BL, P + 1], F32, name='ex', tag='ex')
        sm = pw.tile([BL, 1], F32, name='sm', tag='sm')
        nc.scalar.activation(ex[:], sh[:], AF.Exp, accum_out=sm[:, 0:1])
        rc = pw.tile([BL, 1], F32, name='rc', tag='rc')
        nc.vector.reciprocal(rc[:], sm[:])
        res = pw.tile([BL, P + 1], F32, name='res', tag='res')
        nc.vector.tensor_scalar(res[:], ex[:], rc[:, 0:1], None, op0=ALU.mult)
        nc.sync.dma_start(out=out_d[:], in_=res[:])

    nc.compile()
    return nc


def _get_nc():
    if 'nc' not in _CACHE:
        _CACHE['nc'] = _build_nc()
    return _CACHE['nc']


# ============================ entry point ============================

def _shard_inputs(inputs):
    folded = _host_fold(inputs)
    obs = np.asarray(inputs['observation'], np.float32)
    action = np.asarray(inputs['action'], np.float32)
    obs_t = np.ascontiguousarray(obs.transpose(0, 1, 3, 2)).reshape(B, 150, N)

    in_maps = []
    for i in range(NCORES):
        bs = slice(i * BL, (i + 1) * BL)
        ot = obs_t[bs]
        rh = ot[:, 0:128, :].transpose(1, 0, 2).reshape(128, BL * N)
        rl = ot[:, 128:150, :].transpose(1, 0, 2).reshape(22, BL * N)
        onat = (obs[bs].reshape(BL, C0, 4, NCH, T)
                .transpose(3, 0, 2, 1, 4).reshape(NCH, 4800))
        ats = (action[bs, 1:].reshape(BL, 4, NCH)
               .transpose(2, 1, 0).reshape(NCH, 32))
        wt = folded['wt_const'].copy()
        wt[0:NCH, OT_ATS:OT_ATS + 32] = ats.astype(BF)
        in_maps.append({
            'packA': np.concatenate([folded['pa_const'], rh.astype(BF)], axis=1),
            'pack22': np.concatenate([folded['p22_const'], rl.astype(BF)], axis=1),
            'pdsmall': folded['ps_const'],
            'onat0': np.ascontiguousarray(onat[:, 0:2400]).astype(BF),
            'onat1': np.ascontiguousarray(onat[:, 2400:4800]).astype(BF),
            'attr0': folded['attr'][0], 'attr1': folded['attr'][1],
            'attr2': folded['attr'][2], 'attr3': folded['attr'][3],
            'wtail': wt, 'biasF': folded['biasf'],
        })
    return in_maps


def kernel(**inputs) -> np.ndarray:
    from concourse.bass_utils import run_bass_kernel_spmd

    in_maps = _shard_inputs(inputs)
    nc = _get_nc()
    res = run_bass_kernel_spmd(nc, in_maps, list(range(NCORES)))
    return np.concatenate([np.asarray(r['out'], np.float32)
                           for r in res.results], axis=0)
